# revision 5
# baseline (speedup 1.0000x reference)
"""MixTreeLSTMCell Trainium2 kernel (8 NeuronCores, SPMD) — v2.

Changes vs v1 (420us baseline):
- x^T zero-padded on host from 301 to 384 contraction rows: every matmul is
  a full 128-partition instruction (45-row matmuls measured 1.5x slower
  per moving row than 128-row ones).
- Weight-stationary j-pair inner loops: node tiles are processed in blocks
  of two 512-node tiles; each stationary weight tile is loaded once and
  used for both tiles back-to-back, halving PE weight reloads.
- One 8-bank PSUM pool (bufs=4 of [128,2,512]) rotating through f-gate and
  iou phases, deep enough that ACT drains never stall the PE.
- fp16 outputs (halves store traffic); host converts to fp32.
- Elementwise chain runs in place inside the gates tile; c and h are
  DMA'd straight out of it.
"""

from contextlib import ExitStack

import numpy as np

import concourse.bacc as bacc
import concourse.tile as tile
from concourse import mybir
from concourse import bass_utils

F32 = mybir.dt.float32
FP16 = mybir.dt.float16
NP_FP16 = np.float16

N_NODES = 131072
X = 300
XP = 384              # x rows + ones(bias) row at 300, zero-padded to 3*128
H = 256
CORES = 8
TILE_N = 512          # nodes per matmul tile (max moving free dim)
MACRO = 4 * TILE_N    # nodes per DMA macro tile

TRACE = False
LAST_EXEC_NS = None

_PROGRAM_CACHE = {}


def _round_up(v, m):
    return (v + m - 1) // m * m


def _build_program(T0, T1):
    key = (T0, T1)
    if key in _PROGRAM_CACHE:
        return _PROGRAM_CACHE[key]

    Nc = (T0 + T1) * TILE_N
    nc = bacc.Bacc("TRN2", target_bir_lowering=False, debug=False)

    xT = nc.dram_tensor("xT", [XP, Nc], FP16, kind="ExternalInput").ap()
    hT = nc.dram_tensor("hT", [2 * H, Nc], FP16, kind="ExternalInput").ap()
    cT = nc.dram_tensor("cT", [2 * H, Nc], FP16, kind="ExternalInput").ap()

    WnT = nc.dram_tensor("WnT", [XP, 3 * H], FP16, kind="ExternalInput").ap()
    UnT = nc.dram_tensor("UnT", [2 * H, 3 * H], FP16, kind="ExternalInput").ap()
    UfwT = nc.dram_tensor("UfwT", [2 * H, 2 * H], FP16, kind="ExternalInput").ap()
    WsT = nc.dram_tensor("WsT", [XP, 3 * H], FP16, kind="ExternalInput").ap()
    UsT = nc.dram_tensor("UsT", [H, 3 * H], FP16, kind="ExternalInput").ap()
    UfswT = nc.dram_tensor("UfswT", [H, H], FP16, kind="ExternalInput").ap()

    bias_fn = nc.dram_tensor("bias_fn", [128, 4], F32, kind="ExternalInput").ap()
    bias_fs = nc.dram_tensor("bias_fs", [128, 2], F32, kind="ExternalInput").ap()

    hOT = nc.dram_tensor("hOT", [H, Nc], FP16, kind="ExternalOutput").ap()
    cOT = nc.dram_tensor("cOT", [H, Nc], FP16, kind="ExternalOutput").ap()

    xT_v = xT.rearrange("(ko p) n -> p ko n", p=128)
    hT_v = hT.rearrange("(ko p) n -> p ko n", p=128)
    cT_v = cT.rearrange("(ko p) n -> p ko n", p=128)
    hOT_v = hOT.rearrange("(ko p) n -> p ko n", p=128)
    cOT_v = cOT.rearrange("(ko p) n -> p ko n", p=128)
    WnT_v = WnT.rearrange("(ko p) m -> p ko m", p=128)
    WsT_v = WsT.rearrange("(ko p) m -> p ko m", p=128)
    UnT_v = UnT.rearrange("(ko p) m -> p ko m", p=128)
    UfwT_v = UfwT.rearrange("(ko p) m -> p ko m", p=128)
    UsT_v = UsT.rearrange("(ko p) m -> p ko m", p=128)
    UfswT_v = UfswT.rearrange("(ko p) m -> p ko m", p=128)

    SIG = mybir.ActivationFunctionType.Sigmoid
    TANH = mybir.ActivationFunctionType.Tanh

    with tile.TileContext(nc) as tc, ExitStack() as stack:
        wp = stack.enter_context(tc.tile_pool(name="w", bufs=1))
        io = stack.enter_context(tc.tile_pool(name="io", bufs=2))
        mid = stack.enter_context(tc.tile_pool(name="mid", bufs=2))
        psp = stack.enter_context(tc.tile_pool(name="psp", bufs=4, space="PSUM"))

        # --- resident weights in need order: the two type-1 warmup singles
        # run first (Ufsw, Ws, Us), then type-0 (Ufw, Wn, Un); spread across
        # the three DMA rings so no single ring serializes the head ---
        Ufsw_sb = wp.tile([128, 2, H], FP16)
        nc.sync.dma_start(out=Ufsw_sb, in_=UfswT_v)
        bfs_sb = wp.tile([128, 2], F32)
        nc.sync.dma_start(out=bfs_sb, in_=bias_fs)
        bfn_sb = wp.tile([128, 4], F32)
        nc.gpsimd.dma_start(out=bfn_sb, in_=bias_fn)
        Ws_sb = wp.tile([128, 3, 3 * H], FP16)
        Us_sb = wp.tile([128, 2, 3 * H], FP16)
        nc.scalar.dma_start(out=Ws_sb[:, 0, :], in_=WsT_v[:, 0, :])
        nc.gpsimd.dma_start(out=Ws_sb[:, 1, :], in_=WsT_v[:, 1, :])
        nc.scalar.dma_start(out=Ws_sb[:, 2, :], in_=WsT_v[:, 2, :])
        nc.gpsimd.dma_start(out=Us_sb[:, 0, :], in_=UsT_v[:, 0, :])
        nc.scalar.dma_start(out=Us_sb[:, 1, :], in_=UsT_v[:, 1, :])
        # type-0 weights: allocated here, but their DMAs are emitted only
        # after the warmup singles' data loads so the ring queues serve the
        # head in true need order.
        Ufw_sb = wp.tile([128, 4, 2 * H], FP16)
        Wn_sb = wp.tile([128, 3, 3 * H], FP16)
        Un_sb = wp.tile([128, 4, 3 * H], FP16)
        br0_fired = [False]

        def load_br0_weights():
            if br0_fired[0]:
                return
            br0_fired[0] = True
            nc.sync.dma_start(out=Ufw_sb[:, 0:2, :], in_=UfwT_v[:, 0:2, :])
            nc.gpsimd.dma_start(out=Ufw_sb[:, 2:4, :], in_=UfwT_v[:, 2:4, :])
            nc.sync.dma_start(out=Wn_sb[:, 0, :], in_=WnT_v[:, 0, :])
            nc.scalar.dma_start(out=Wn_sb[:, 1, :], in_=WnT_v[:, 1, :])
            nc.gpsimd.dma_start(out=Wn_sb[:, 2, :], in_=WnT_v[:, 2, :])
            nc.sync.dma_start(out=Un_sb[:, 0, :], in_=UnT_v[:, 0, :])
            nc.scalar.dma_start(out=Un_sb[:, 1, :], in_=UnT_v[:, 1, :])
            nc.gpsimd.dma_start(out=Un_sb[:, 2, :], in_=UnT_v[:, 2, :])
            nc.sync.dma_start(out=Un_sb[:, 3, :], in_=UnT_v[:, 3, :])

        def do_block(br, xt, ht, ct, htild, j0, n0, jw):
            """Process jw (1 or 2) node tiles starting at macro tile j0.
            n0 = DRAM node offset of the block start."""
            w = jw * TILE_N
            cols = [slice((j0 + j) * TILE_N, (j0 + j + 1) * TILE_N)
                    for j in range(jw)]
            span = slice(j0 * TILE_N, j0 * TILE_N + w)

            f_full = mid.tile([128, 4, 2 * TILE_N], FP16, tag="f", name="f")
            f = f_full[:, :, :w]

            # --- forget gates ---
            if br == 0:
                for m in range(4):
                    ps = psp.tile([128, 2, TILE_N], F32, tag="ps", name="ps")
                    for j in range(jw):
                        for k in range(4):
                            nc.tensor.matmul(
                                ps[:, j, :],
                                Ufw_sb[:, k, 128 * m : 128 * (m + 1)],
                                ht[:, k, cols[j]],
                                start=(k == 0),
                                stop=(k == 3),
                            )
                    nc.scalar.activation(
                        out=f[:, m, :],
                        in_=ps[:, :jw, :].rearrange("p a b -> p (a b)"),
                        func=SIG,
                        bias=bfn_sb[:, m : m + 1],
                    )
            else:
                for child in range(2):
                    for m in range(2):
                        ps = psp.tile([128, 2, TILE_N], F32, tag="ps", name="ps")
                        for j in range(jw):
                            for k in range(2):
                                nc.tensor.matmul(
                                    ps[:, j, :],
                                    Ufsw_sb[:, k, 128 * m : 128 * (m + 1)],
                                    ht[:, 2 * child + k, cols[j]],
                                    start=(k == 0),
                                    stop=(k == 1),
                                )
                        nc.scalar.activation(
                            out=f[:, 2 * child + m, :],
                            in_=ps[:, :jw, :].rearrange("p a b -> p (a b)"),
                            func=SIG,
                            bias=bfs_sb[:, m : m + 1],
                        )

            # prod = f * c_child (in place); cred = child0 + child1
            nc.vector.tensor_mul(out=f, in0=f, in1=ct[:, :, span])
            cred_full = mid.tile([128, 2, 2 * TILE_N], F32, tag="cred", name="cred")
            cred = cred_full[:, :, :w]
            nc.vector.tensor_add(out=cred, in0=f[:, 0:2, :], in1=f[:, 2:4, :])

            # --- iou gates: 3 m-pair phases, weights shared across the
            # block's tiles ---
            gates_full = mid.tile([128, 6, 2 * TILE_N], FP16, tag="g", name="g")
            gates = gates_full[:, :, :w]
            for mp in range(3):
                pss = [
                    psp.tile([128, 2, TILE_N], F32, tag="ps", name="ps")
                    for _ in range(jw)
                ]
                for j in range(jw):
                    for m2 in range(2):
                        m = 2 * mp + m2
                        ms = slice(128 * m, 128 * (m + 1))
                        if br == 0:
                            for k in range(3):
                                nc.tensor.matmul(
                                    pss[j][:, m2, :], Wn_sb[:, k, ms],
                                    xt[:, k, cols[j]],
                                    start=(k == 0), stop=False,
                                )
                            for k in range(4):
                                nc.tensor.matmul(
                                    pss[j][:, m2, :], Un_sb[:, k, ms],
                                    ht[:, k, cols[j]],
                                    start=False, stop=(k == 3),
                                )
                        else:
                            for k in range(3):
                                nc.tensor.matmul(
                                    pss[j][:, m2, :], Ws_sb[:, k, ms],
                                    xt[:, k, cols[j]],
                                    start=(k == 0), stop=False,
                                )
                            for k in range(2):
                                nc.tensor.matmul(
                                    pss[j][:, m2, :], Us_sb[:, k, ms],
                                    htild[:, k, cols[j]],
                                    start=False, stop=(k == 1),
                                )
                for j in range(jw):
                    nc.scalar.activation(
                        out=gates[:, 2 * mp : 2 * mp + 2,
                                  j * TILE_N : (j + 1) * TILE_N],
                        in_=pss[j],
                        func=TANH if mp == 2 else SIG,
                    )

            # c = sig(i)*tanh(u) + cred (into gates[:,0:2])
            nc.vector.tensor_mul(
                out=gates[:, 0:2, :], in0=gates[:, 0:2, :], in1=gates[:, 4:6, :]
            )
            nc.vector.tensor_add(out=gates[:, 0:2, :], in0=gates[:, 0:2, :], in1=cred)
            nc.gpsimd.dma_start(out=cOT_v[:, :, n0 : n0 + w], in_=gates[:, 0:2, :])
            # h = sig(o)*tanh(c) (tanh into gates[:,4:6], h into gates[:,2:4])
            nc.scalar.activation(out=gates[:, 4:6, :], in_=gates[:, 0:2, :], func=TANH)
            nc.vector.tensor_mul(
                out=gates[:, 2:4, :], in0=gates[:, 2:4, :], in1=gates[:, 4:6, :]
            )
            nc.gpsimd.dma_start(out=hOT_v[:, :, n0 : n0 + w], in_=gates[:, 2:4, :])

        # --- macro schedule: two type-1 singles (cheap weight prefix
        # unblocks the PE early), then all type-0, then remaining type-1 ---
        s1 = min(2, T1)
        entries = []
        if s1:
            entries.append((1, T0 * TILE_N, s1, True))
        if T0:
            entries.append((0, 0, T0, False))
        if T1 - s1:
            entries.append((1, (T0 + s1) * TILE_N, T1 - s1, False))
        for ei, (br, base, T, singles) in enumerate(entries):
            starts = []
            g = 0
            while g < T:
                nt = 1 if singles else min(4, T - g)
                starts.append((g, nt))
                g += nt
            for gi, (g, nt) in enumerate(starts):
                n0 = base + g * TILE_N
                w = nt * TILE_N
                head = ei == 0 and gi == 0
                xt_full = io.tile([128, 3, MACRO], FP16, tag="xt", name="xt")
                xt = xt_full[:, :, :w]
                xeng = nc.gpsimd if head else nc.sync
                for k in range(3):
                    xeng.dma_start(out=xt[:, k, :], in_=xT_v[:, k, n0 : n0 + w])
                ht_full = io.tile([128, 4, MACRO], FP16, tag="ht", name="ht")
                ht = ht_full[:, :, :w]
                nc.sync.dma_start(out=ht, in_=hT_v[:, :, n0 : n0 + w])
                ct_full = io.tile([128, 4, MACRO], FP16, tag="ct", name="ct")
                ct = ct_full[:, :, :w]
                (nc.gpsimd if head else nc.scalar).dma_start(
                    out=ct, in_=cT_v[:, :, n0 : n0 + w]
                )
                if (ei == 0 and gi == len(starts) - 1) or br == 0:
                    load_br0_weights()

                htild = None
                if br == 1:
                    htild_full = io.tile(
                        [128, 2, MACRO], FP16, tag="htild", name="htild"
                    )
                    htild = htild_full[:, :, :w]
                    nc.vector.tensor_add(
                        out=htild, in0=ht[:, 0:2, :], in1=ht[:, 2:4, :]
                    )

                last = ei == len(entries) - 1 and (g, nt) == starts[-1]
                j0 = 0
                while j0 < nt:
                    jw = 1 if (last and j0 >= nt - 2) else min(2, nt - j0)
                    do_block(br, xt, ht, ct, htild, j0, n0 + j0 * TILE_N, jw)
                    j0 += jw

    nc.compile()
    _PROGRAM_CACHE[key] = nc
    return nc


def kernel(x, h_child, c_child, t, W_iou, U_iou, b_iou, U_f_w, U_f_b,
           W_iou_s, U_iou_s, b_iou_s, U_f_s_w, U_f_s_b):
    global LAST_EXEC_NS
    x = np.asarray(x, dtype=np.float32)
    h_child = np.asarray(h_child, dtype=np.float32)
    c_child = np.asarray(c_child, dtype=np.float32)
    t = np.asarray(t)
    n = x.shape[0]

    # --- host partition: equal per-core type counts, padded to tiles ---
    idx0 = np.flatnonzero(t == 0)
    idx1 = np.flatnonzero(t != 0)
    n0, n1 = len(idx0), len(idx1)

    def pad_split(idx, cnt):
        if cnt == 0:
            return np.zeros((CORES, 0), dtype=np.int64), 0
        per = _round_up(-(-cnt // CORES), TILE_N)
        padded = np.concatenate(
            [idx, np.full(CORES * per - cnt, idx[-1], dtype=idx.dtype)]
        )
        return padded.reshape(CORES, per).astype(np.int64), per

    chunks0, P0 = pad_split(idx0, n0)
    chunks1, P1 = pad_split(idx1, n1)
    T0, T1 = P0 // TILE_N, P1 // TILE_N

    nc = _build_program(T0, T1)

    hc2 = h_child.reshape(n, 2 * H)
    cc2 = c_child.reshape(n, 2 * H)

    def bias_tile(v, m):
        return np.ascontiguousarray(
            np.asarray(v, np.float32).reshape(-1)[: 128 * m].reshape(m, 128).T
        )

    def w_with_bias(W, b):
        # [XP, 768] = W^T with bias as row 300, zero rows 301..383
        out = np.zeros((XP, 3 * H), dtype=NP_FP16)
        out[:X] = np.asarray(W, np.float32).T.astype(NP_FP16)
        out[X] = np.asarray(b, np.float32).reshape(-1).astype(NP_FP16)
        return out

    wmap = {
        "WnT": w_with_bias(W_iou, b_iou),
        "UnT": np.ascontiguousarray(np.asarray(U_iou, np.float32).T).astype(NP_FP16),
        "UfwT": np.ascontiguousarray(np.asarray(U_f_w, np.float32).T).astype(NP_FP16),
        "WsT": w_with_bias(W_iou_s, b_iou_s),
        "UsT": np.ascontiguousarray(np.asarray(U_iou_s, np.float32).T).astype(NP_FP16),
        "UfswT": np.ascontiguousarray(np.asarray(U_f_s_w, np.float32).T).astype(NP_FP16),
        "bias_fn": bias_tile(U_f_b, 4),
        "bias_fs": bias_tile(U_f_s_b, 2),
    }

    in_maps = []
    for i in range(CORES):
        I = np.concatenate([chunks0[i], chunks1[i]])
        xTi = np.zeros((XP, len(I)), dtype=NP_FP16)
        xTi[:X] = x[I].T.astype(NP_FP16)
        xTi[X] = 1.0
        m = dict(wmap)
        m["xT"] = xTi
        m["hT"] = hc2[I].T.astype(NP_FP16)
        m["cT"] = cc2[I].T.astype(NP_FP16)
        in_maps.append(m)

    res = bass_utils.run_bass_kernel_spmd(
        nc, in_maps, core_ids=list(range(CORES)), trace=TRACE
    )
    LAST_EXEC_NS = res.exec_time_ns

    # --- scatter back ---
    h_out = np.empty((n, H), dtype=np.float32)
    c_out = np.empty((n, H), dtype=np.float32)
    if n0:
        h0 = np.concatenate(
            [res.results[i]["hOT"][:, :P0].T.astype(np.float32) for i in range(CORES)]
        )
        c0 = np.concatenate(
            [res.results[i]["cOT"][:, :P0].T.astype(np.float32) for i in range(CORES)]
        )
        h_out[idx0] = h0[:n0]
        c_out[idx0] = c0[:n0]
    if n1:
        h1 = np.concatenate(
            [res.results[i]["hOT"][:, P0:].T.astype(np.float32) for i in range(CORES)]
        )
        c1 = np.concatenate(
            [res.results[i]["cOT"][:, P0:].T.astype(np.float32) for i in range(CORES)]
        )
        h_out[idx1] = h1[:n1]
        c_out[idx1] = c1[:n1]
    return h_out, c_out


# revision 6
# speedup vs baseline: 1.1977x; 1.1977x over previous
"""MixTreeLSTMCell Trainium2 kernel (8 NeuronCores, SPMD) — v2.

Changes vs v1 (420us baseline):
- x^T zero-padded on host from 301 to 384 contraction rows: every matmul is
  a full 128-partition instruction (45-row matmuls measured 1.5x slower
  per moving row than 128-row ones).
- Weight-stationary j-pair inner loops: node tiles are processed in blocks
  of two 512-node tiles; each stationary weight tile is loaded once and
  used for both tiles back-to-back, halving PE weight reloads.
- One 8-bank PSUM pool (bufs=4 of [128,2,512]) rotating through f-gate and
  iou phases, deep enough that ACT drains never stall the PE.
- fp16 outputs (halves store traffic); host converts to fp32.
- Elementwise chain runs in place inside the gates tile; c and h are
  DMA'd straight out of it.
"""

from contextlib import ExitStack

import numpy as np

import concourse.bacc as bacc
import concourse.tile as tile
from concourse import mybir
from concourse import bass_utils

F32 = mybir.dt.float32
FP16 = mybir.dt.float16
NP_FP16 = np.float16

N_NODES = 131072
X = 300
XP = 384              # x rows + ones(bias) row at 300, zero-padded to 3*128
H = 256
CORES = 8
TILE_N = 512          # nodes per matmul tile (max moving free dim)
MACRO = 4 * TILE_N    # nodes per DMA macro tile

TRACE = False
LAST_EXEC_NS = None

_PROGRAM_CACHE = {}


def _round_up(v, m):
    return (v + m - 1) // m * m


def _build_program(T0h, T1h):
    """T0h/T1h: per-core type-0/1 node counts in 256-node half-tile units."""
    key = (T0h, T1h)
    if key in _PROGRAM_CACHE:
        return _PROGRAM_CACHE[key]

    Nc = (T0h + T1h) * (TILE_N // 2)
    nc = bacc.Bacc("TRN2", target_bir_lowering=False, debug=False)

    xT = nc.dram_tensor("xT", [XP, Nc], FP16, kind="ExternalInput").ap()
    hT = nc.dram_tensor("hT", [2 * H, Nc], FP16, kind="ExternalInput").ap()
    cT = nc.dram_tensor("cT", [2 * H, Nc], FP16, kind="ExternalInput").ap()

    WnT = nc.dram_tensor("WnT", [XP, 3 * H], FP16, kind="ExternalInput").ap()
    UnT = nc.dram_tensor("UnT", [2 * H, 3 * H], FP16, kind="ExternalInput").ap()
    UfwT = nc.dram_tensor("UfwT", [2 * H, 2 * H], FP16, kind="ExternalInput").ap()
    WsT = nc.dram_tensor("WsT", [XP, 3 * H], FP16, kind="ExternalInput").ap()
    UsT = nc.dram_tensor("UsT", [H, 3 * H], FP16, kind="ExternalInput").ap()
    UfswT = nc.dram_tensor("UfswT", [H, H], FP16, kind="ExternalInput").ap()

    bias_fn = nc.dram_tensor("bias_fn", [128, 4], F32, kind="ExternalInput").ap()
    bias_fs = nc.dram_tensor("bias_fs", [128, 2], F32, kind="ExternalInput").ap()

    hOT = nc.dram_tensor("hOT", [H, Nc], FP16, kind="ExternalOutput").ap()
    cOT = nc.dram_tensor("cOT", [H, Nc], FP16, kind="ExternalOutput").ap()

    xT_v = xT.rearrange("(ko p) n -> p ko n", p=128)
    hT_v = hT.rearrange("(ko p) n -> p ko n", p=128)
    cT_v = cT.rearrange("(ko p) n -> p ko n", p=128)
    hOT_v = hOT.rearrange("(ko p) n -> p ko n", p=128)
    cOT_v = cOT.rearrange("(ko p) n -> p ko n", p=128)
    WnT_v = WnT.rearrange("(ko p) m -> p ko m", p=128)
    WsT_v = WsT.rearrange("(ko p) m -> p ko m", p=128)
    UnT_v = UnT.rearrange("(ko p) m -> p ko m", p=128)
    UfwT_v = UfwT.rearrange("(ko p) m -> p ko m", p=128)
    UsT_v = UsT.rearrange("(ko p) m -> p ko m", p=128)
    UfswT_v = UfswT.rearrange("(ko p) m -> p ko m", p=128)

    SIG = mybir.ActivationFunctionType.Sigmoid
    TANH = mybir.ActivationFunctionType.Tanh

    with tile.TileContext(nc) as tc, ExitStack() as stack:
        wp = stack.enter_context(tc.tile_pool(name="w", bufs=1))
        io = stack.enter_context(tc.tile_pool(name="io", bufs=2))
        mid = stack.enter_context(tc.tile_pool(name="mid", bufs=2))
        psp = stack.enter_context(tc.tile_pool(name="psp", bufs=4, space="PSUM"))

        # --- resident weights in need order: the two type-1 warmup singles
        # run first (Ufsw, Ws, Us), then type-0 (Ufw, Wn, Un); spread across
        # the three DMA rings so no single ring serializes the head ---
        Ufsw_sb = wp.tile([128, 2, H], FP16)
        nc.sync.dma_start(out=Ufsw_sb, in_=UfswT_v)
        bfs_sb = wp.tile([128, 2], F32)
        nc.sync.dma_start(out=bfs_sb, in_=bias_fs)
        bfn_sb = wp.tile([128, 4], F32)
        nc.gpsimd.dma_start(out=bfn_sb, in_=bias_fn)
        Ws_sb = wp.tile([128, 3, 3 * H], FP16)
        Us_sb = wp.tile([128, 2, 3 * H], FP16)
        nc.scalar.dma_start(out=Ws_sb[:, 0, :], in_=WsT_v[:, 0, :])
        nc.gpsimd.dma_start(out=Ws_sb[:, 1, :], in_=WsT_v[:, 1, :])
        nc.scalar.dma_start(out=Ws_sb[:, 2, :], in_=WsT_v[:, 2, :])
        nc.gpsimd.dma_start(out=Us_sb[:, 0, :], in_=UsT_v[:, 0, :])
        nc.scalar.dma_start(out=Us_sb[:, 1, :], in_=UsT_v[:, 1, :])
        # type-0 weights: allocated here, but their DMAs are emitted only
        # after the warmup singles' data loads so the ring queues serve the
        # head in true need order.
        Ufw_sb = wp.tile([128, 4, 2 * H], FP16)
        Wn_sb = wp.tile([128, 3, 3 * H], FP16)
        Un_sb = wp.tile([128, 4, 3 * H], FP16)
        br0_fired = [False]

        def load_br0_weights():
            if br0_fired[0]:
                return
            br0_fired[0] = True
            nc.sync.dma_start(out=Ufw_sb[:, 0:2, :], in_=UfwT_v[:, 0:2, :])
            nc.gpsimd.dma_start(out=Ufw_sb[:, 2:4, :], in_=UfwT_v[:, 2:4, :])
            nc.sync.dma_start(out=Wn_sb[:, 0, :], in_=WnT_v[:, 0, :])
            nc.scalar.dma_start(out=Wn_sb[:, 1, :], in_=WnT_v[:, 1, :])
            nc.gpsimd.dma_start(out=Wn_sb[:, 2, :], in_=WnT_v[:, 2, :])
            nc.sync.dma_start(out=Un_sb[:, 0, :], in_=UnT_v[:, 0, :])
            nc.scalar.dma_start(out=Un_sb[:, 1, :], in_=UnT_v[:, 1, :])
            nc.gpsimd.dma_start(out=Un_sb[:, 2, :], in_=UnT_v[:, 2, :])
            nc.sync.dma_start(out=Un_sb[:, 3, :], in_=UnT_v[:, 3, :])

        def do_block(br, xt, ht, ct, htild, j0, n0, jw, ncol=TILE_N):
            """Process jw (1 or 2) node tiles starting at macro tile j0.
            n0 = DRAM node offset; ncol<=512 nodes per tile (last tile of a
            segment may be a 256-node half tile)."""
            w = (jw - 1) * TILE_N + ncol
            cols = [slice((j0 + j) * TILE_N,
                          (j0 + j) * TILE_N + (TILE_N if j < jw - 1 else ncol))
                    for j in range(jw)]
            span = slice(j0 * TILE_N, j0 * TILE_N + w)

            f_full = mid.tile([128, 4, 2 * TILE_N], FP16, tag="f", name="f")
            f = f_full[:, :, :w]

            # --- forget gates ---
            if br == 0:
                for m in range(4):
                    ps = psp.tile([128, 2, TILE_N], F32, tag="ps", name="ps")
                    for j in range(jw):
                        nj = cols[j].stop - cols[j].start
                        for k in range(4):
                            nc.tensor.matmul(
                                ps[:, j, :nj],
                                Ufw_sb[:, k, 128 * m : 128 * (m + 1)],
                                ht[:, k, cols[j]],
                                start=(k == 0),
                                stop=(k == 3),
                            )
                    if ncol == TILE_N:
                        nc.scalar.activation(
                            out=f[:, m, :],
                            in_=ps[:, :jw, :].rearrange("p a b -> p (a b)"),
                            func=SIG,
                            bias=bfn_sb[:, m : m + 1],
                        )
                    else:
                        for j in range(jw):
                            nj = cols[j].stop - cols[j].start
                            nc.scalar.activation(
                                out=f[:, m, j * TILE_N : j * TILE_N + nj],
                                in_=ps[:, j, :nj],
                                func=SIG,
                                bias=bfn_sb[:, m : m + 1],
                            )
            else:
                for child in range(2):
                    for m in range(2):
                        ps = psp.tile([128, 2, TILE_N], F32, tag="ps", name="ps")
                        for j in range(jw):
                            nj = cols[j].stop - cols[j].start
                            for k in range(2):
                                nc.tensor.matmul(
                                    ps[:, j, :nj],
                                    Ufsw_sb[:, k, 128 * m : 128 * (m + 1)],
                                    ht[:, 2 * child + k, cols[j]],
                                    start=(k == 0),
                                    stop=(k == 1),
                                )
                        if ncol == TILE_N:
                            nc.scalar.activation(
                                out=f[:, 2 * child + m, :],
                                in_=ps[:, :jw, :].rearrange("p a b -> p (a b)"),
                                func=SIG,
                                bias=bfs_sb[:, m : m + 1],
                            )
                        else:
                            for j in range(jw):
                                nj = cols[j].stop - cols[j].start
                                nc.scalar.activation(
                                    out=f[:, 2 * child + m,
                                          j * TILE_N : j * TILE_N + nj],
                                    in_=ps[:, j, :nj],
                                    func=SIG,
                                    bias=bfs_sb[:, m : m + 1],
                                )

            # prod = f * c_child (in place); cred = child0 + child1
            nc.vector.tensor_mul(out=f, in0=f, in1=ct[:, :, span])
            cred_full = mid.tile([128, 2, 2 * TILE_N], F32, tag="cred", name="cred")
            cred = cred_full[:, :, :w]
            nc.vector.tensor_add(out=cred, in0=f[:, 0:2, :], in1=f[:, 2:4, :])

            # --- iou gates: 3 m-pair phases, weights shared across the
            # block's tiles ---
            gates_full = mid.tile([128, 6, 2 * TILE_N], FP16, tag="g", name="g")
            gates = gates_full[:, :, :w]
            for mp in range(3):
                pss = [
                    psp.tile([128, 2, TILE_N], F32, tag="ps", name="ps")
                    for _ in range(jw)
                ]
                for j in range(jw):
                    nj = cols[j].stop - cols[j].start
                    for m2 in range(2):
                        m = 2 * mp + m2
                        ms = slice(128 * m, 128 * (m + 1))
                        if br == 0:
                            for k in range(3):
                                nc.tensor.matmul(
                                    pss[j][:, m2, :nj], Wn_sb[:, k, ms],
                                    xt[:, k, cols[j]],
                                    start=(k == 0), stop=False,
                                )
                            for k in range(4):
                                nc.tensor.matmul(
                                    pss[j][:, m2, :nj], Un_sb[:, k, ms],
                                    ht[:, k, cols[j]],
                                    start=False, stop=(k == 3),
                                )
                        else:
                            for k in range(3):
                                nc.tensor.matmul(
                                    pss[j][:, m2, :nj], Ws_sb[:, k, ms],
                                    xt[:, k, cols[j]],
                                    start=(k == 0), stop=False,
                                )
                            for k in range(2):
                                nc.tensor.matmul(
                                    pss[j][:, m2, :nj], Us_sb[:, k, ms],
                                    htild[:, k, cols[j]],
                                    start=False, stop=(k == 1),
                                )
                for j in range(jw):
                    nj = cols[j].stop - cols[j].start
                    nc.scalar.activation(
                        out=gates[:, 2 * mp : 2 * mp + 2,
                                  j * TILE_N : j * TILE_N + nj],
                        in_=pss[j][:, :, :nj],
                        func=TANH if mp == 2 else SIG,
                    )

            # c = sig(i)*tanh(u) + cred (into gates[:,0:2])
            nc.vector.tensor_mul(
                out=gates[:, 0:2, :], in0=gates[:, 0:2, :], in1=gates[:, 4:6, :]
            )
            nc.vector.tensor_add(out=gates[:, 0:2, :], in0=gates[:, 0:2, :], in1=cred)
            nc.gpsimd.dma_start(out=cOT_v[:, :, n0 : n0 + w], in_=gates[:, 0:2, :])
            # h = sig(o)*tanh(c) (tanh into gates[:,4:6], h into gates[:,2:4])
            nc.scalar.activation(out=gates[:, 4:6, :], in_=gates[:, 0:2, :], func=TANH)
            nc.vector.tensor_mul(
                out=gates[:, 2:4, :], in0=gates[:, 2:4, :], in1=gates[:, 4:6, :]
            )
            nc.gpsimd.dma_start(out=hOT_v[:, :, n0 : n0 + w], in_=gates[:, 2:4, :])

        # --- macro schedule: two type-1 singles (cheap weight prefix
        # unblocks the PE early), then all type-0, then remaining type-1.
        # Counts are in 256-node halves; a segment's odd tail half-tile is
        # processed as a 256-node tile at the segment end. ---
        HALF = TILE_N // 2
        T0, half0 = divmod(T0h, 2)
        T1, half1 = divmod(T1h, 2)
        s1 = min(2, T1)
        base1 = T0h * HALF
        entries = []
        if s1:
            entries.append((1, base1, s1, half1, True))
        if T0 or half0:
            entries.append((0, 0, T0, half0, False))
        if (T1 - s1) or half1:
            entries.append((1, base1 + s1 * TILE_N, T1 - s1, half1, False))
        for ei, (br, base, T, half, singles) in enumerate(entries):
            starts = []
            g = 0
            while g < T:
                nt = 1 if singles else min(4, T - g)
                starts.append((g, nt))
                g += nt
            if half and not singles:
                # tack the 256-node tail tile onto the last macro (or its own)
                if starts and starts[-1][1] < 4:
                    starts[-1] = (starts[-1][0], starts[-1][1] + 0.5)
                else:
                    starts.append((g, 0.5))
            for gi, (g, ntf) in enumerate(starts):
                nt = int(ntf)
                tail = ntf != nt          # has a trailing 256-node half tile
                n0 = base + g * TILE_N
                w = nt * TILE_N + (HALF if tail else 0)
                nt_eff = nt + (1 if tail else 0)
                head = ei == 0 and gi == 0
                xt_full = io.tile([128, 3, MACRO], FP16, tag="xt", name="xt")
                xt = xt_full[:, :, :w]
                xeng = nc.gpsimd if head else nc.sync
                for k in range(3):
                    xeng.dma_start(out=xt[:, k, :], in_=xT_v[:, k, n0 : n0 + w])
                ht_full = io.tile([128, 4, MACRO], FP16, tag="ht", name="ht")
                ht = ht_full[:, :, :w]
                nc.sync.dma_start(out=ht, in_=hT_v[:, :, n0 : n0 + w])
                ct_full = io.tile([128, 4, MACRO], FP16, tag="ct", name="ct")
                ct = ct_full[:, :, :w]
                (nc.gpsimd if head else nc.scalar).dma_start(
                    out=ct, in_=cT_v[:, :, n0 : n0 + w]
                )
                if (ei == 0 and gi == len(starts) - 1) or br == 0:
                    load_br0_weights()

                htild = None
                if br == 1:
                    htild_full = io.tile(
                        [128, 2, MACRO], FP16, tag="htild", name="htild"
                    )
                    htild = htild_full[:, :, :w]
                    nc.vector.tensor_add(
                        out=htild, in0=ht[:, 0:2, :], in1=ht[:, 2:4, :]
                    )

                last = ei == len(entries) - 1 and gi == len(starts) - 1
                j0 = 0
                while j0 < nt_eff:
                    jw = 1 if (last and j0 >= nt_eff - 2) else min(2, nt_eff - j0)
                    ncol = HALF if (tail and j0 + jw == nt_eff) else TILE_N
                    do_block(br, xt, ht, ct, htild, j0, n0 + j0 * TILE_N, jw,
                             ncol)
                    j0 += jw

    nc.compile()
    _PROGRAM_CACHE[key] = nc
    return nc


def kernel(x, h_child, c_child, t, W_iou, U_iou, b_iou, U_f_w, U_f_b,
           W_iou_s, U_iou_s, b_iou_s, U_f_s_w, U_f_s_b):
    global LAST_EXEC_NS
    x = np.asarray(x, dtype=np.float32)
    h_child = np.asarray(h_child, dtype=np.float32)
    c_child = np.asarray(c_child, dtype=np.float32)
    t = np.asarray(t)
    n = x.shape[0]

    # --- host partition: equal per-core type counts, padded to tiles ---
    idx0 = np.flatnonzero(t == 0)
    idx1 = np.flatnonzero(t != 0)
    n0, n1 = len(idx0), len(idx1)

    def pad_split(idx, cnt):
        if cnt == 0:
            return np.zeros((CORES, 0), dtype=np.int64), 0
        per = _round_up(-(-cnt // CORES), TILE_N // 2)
        padded = np.concatenate(
            [idx, np.full(CORES * per - cnt, idx[-1], dtype=idx.dtype)]
        )
        return padded.reshape(CORES, per).astype(np.int64), per

    chunks0, P0 = pad_split(idx0, n0)
    chunks1, P1 = pad_split(idx1, n1)

    nc = _build_program(P0 // (TILE_N // 2), P1 // (TILE_N // 2))

    hc2 = h_child.reshape(n, 2 * H)
    cc2 = c_child.reshape(n, 2 * H)

    def bias_tile(v, m):
        return np.ascontiguousarray(
            np.asarray(v, np.float32).reshape(-1)[: 128 * m].reshape(m, 128).T
        )

    def w_with_bias(W, b):
        # [XP, 768] = W^T with bias as row 300, zero rows 301..383
        out = np.zeros((XP, 3 * H), dtype=NP_FP16)
        out[:X] = np.asarray(W, np.float32).T.astype(NP_FP16)
        out[X] = np.asarray(b, np.float32).reshape(-1).astype(NP_FP16)
        return out

    wmap = {
        "WnT": w_with_bias(W_iou, b_iou),
        "UnT": np.ascontiguousarray(np.asarray(U_iou, np.float32).T).astype(NP_FP16),
        "UfwT": np.ascontiguousarray(np.asarray(U_f_w, np.float32).T).astype(NP_FP16),
        "WsT": w_with_bias(W_iou_s, b_iou_s),
        "UsT": np.ascontiguousarray(np.asarray(U_iou_s, np.float32).T).astype(NP_FP16),
        "UfswT": np.ascontiguousarray(np.asarray(U_f_s_w, np.float32).T).astype(NP_FP16),
        "bias_fn": bias_tile(U_f_b, 4),
        "bias_fs": bias_tile(U_f_s_b, 2),
    }

    in_maps = []
    for i in range(CORES):
        I = np.concatenate([chunks0[i], chunks1[i]])
        xTi = np.zeros((XP, len(I)), dtype=NP_FP16)
        xTi[:X] = x[I].T.astype(NP_FP16)
        xTi[X] = 1.0
        m = dict(wmap)
        m["xT"] = xTi
        m["hT"] = hc2[I].T.astype(NP_FP16)
        m["cT"] = cc2[I].T.astype(NP_FP16)
        in_maps.append(m)

    res = bass_utils.run_bass_kernel_spmd(
        nc, in_maps, core_ids=list(range(CORES)), trace=TRACE
    )
    LAST_EXEC_NS = res.exec_time_ns

    # --- scatter back ---
    h_out = np.empty((n, H), dtype=np.float32)
    c_out = np.empty((n, H), dtype=np.float32)
    if n0:
        h0 = np.concatenate(
            [res.results[i]["hOT"][:, :P0].T.astype(np.float32) for i in range(CORES)]
        )
        c0 = np.concatenate(
            [res.results[i]["cOT"][:, :P0].T.astype(np.float32) for i in range(CORES)]
        )
        h_out[idx0] = h0[:n0]
        c_out[idx0] = c0[:n0]
    if n1:
        h1 = np.concatenate(
            [res.results[i]["hOT"][:, P0:].T.astype(np.float32) for i in range(CORES)]
        )
        c1 = np.concatenate(
            [res.results[i]["cOT"][:, P0:].T.astype(np.float32) for i in range(CORES)]
        )
        h_out[idx1] = h1[:n1]
        c_out[idx1] = c1[:n1]
    return h_out, c_out


# revision 7
# speedup vs baseline: 1.1997x; 1.0017x over previous
"""MixTreeLSTMCell Trainium2 kernel (8 NeuronCores, SPMD).

The cell evaluates one of two branches per node (t in {0,1}); the host
partitions nodes by type so each core runs two static branch segments with
no per-node select, on feature-major fp16 operands. The device program is
tensor-engine-bound (~1574 fp16 matmuls/core at ~218ns), so the design
keeps the PE gap-free:
- x^T zero-padded on host from 301 to 384 contraction rows: every matmul
  is a full 128-partition instruction (sub-128-row matmuls measured 1.5x
  slower per moving row on HW).
- Node tiles processed in blocks of two 512-node tiles with bank-stable
  PSUM targeting (runs of consecutive matmuls into one bank), rotating
  through one 8-bank PSUM pool deep enough that ACT drains never stall
  the PE.
- Macro schedule: two type-1 single-tile warmups first (their small
  f/iou weights load fastest, unblocking the PE at ~10us), then all
  type-0 macros, then the remaining type-1; type-0 weights stream in
  lazily behind the warmups' data in ring-queue need order.
- Per-core type counts padded to 256-node halves (segment tails run one
  256-wide tile) to minimize padded compute.
- fp16 outputs; iou bias folded into the matmul via a ones row; f-gate
  biases applied by the scalar engine's free bias; the elementwise chain
  runs in place inside the gates tile and h/c are DMA'd straight out of
  it on the SWDGE ring.
"""

from contextlib import ExitStack

import numpy as np

import concourse.bacc as bacc
import concourse.tile as tile
from concourse import mybir
from concourse import bass_utils

F32 = mybir.dt.float32
FP16 = mybir.dt.float16
NP_FP16 = np.float16

N_NODES = 131072
X = 300
XP = 384              # x rows + ones(bias) row at 300, zero-padded to 3*128
H = 256
CORES = 8
TILE_N = 512          # nodes per matmul tile (max moving free dim)
MACRO = 4 * TILE_N    # nodes per DMA macro tile

TRACE = False
LAST_EXEC_NS = None

_PROGRAM_CACHE = {}


def _round_up(v, m):
    return (v + m - 1) // m * m


def _build_program(T0h, T1h):
    """T0h/T1h: per-core type-0/1 node counts in 256-node half-tile units."""
    key = (T0h, T1h)
    if key in _PROGRAM_CACHE:
        return _PROGRAM_CACHE[key]

    Nc = (T0h + T1h) * (TILE_N // 2)
    nc = bacc.Bacc("TRN2", target_bir_lowering=False, debug=False)

    xT = nc.dram_tensor("xT", [XP, Nc], FP16, kind="ExternalInput").ap()
    hT = nc.dram_tensor("hT", [2 * H, Nc], FP16, kind="ExternalInput").ap()
    cT = nc.dram_tensor("cT", [2 * H, Nc], FP16, kind="ExternalInput").ap()

    WnT = nc.dram_tensor("WnT", [XP, 3 * H], FP16, kind="ExternalInput").ap()
    UnT = nc.dram_tensor("UnT", [2 * H, 3 * H], FP16, kind="ExternalInput").ap()
    UfwT = nc.dram_tensor("UfwT", [2 * H, 2 * H], FP16, kind="ExternalInput").ap()
    WsT = nc.dram_tensor("WsT", [XP, 3 * H], FP16, kind="ExternalInput").ap()
    UsT = nc.dram_tensor("UsT", [H, 3 * H], FP16, kind="ExternalInput").ap()
    UfswT = nc.dram_tensor("UfswT", [H, H], FP16, kind="ExternalInput").ap()

    bias_fn = nc.dram_tensor("bias_fn", [128, 4], F32, kind="ExternalInput").ap()
    bias_fs = nc.dram_tensor("bias_fs", [128, 2], F32, kind="ExternalInput").ap()

    hOT = nc.dram_tensor("hOT", [H, Nc], FP16, kind="ExternalOutput").ap()
    cOT = nc.dram_tensor("cOT", [H, Nc], FP16, kind="ExternalOutput").ap()

    xT_v = xT.rearrange("(ko p) n -> p ko n", p=128)
    hT_v = hT.rearrange("(ko p) n -> p ko n", p=128)
    cT_v = cT.rearrange("(ko p) n -> p ko n", p=128)
    hOT_v = hOT.rearrange("(ko p) n -> p ko n", p=128)
    cOT_v = cOT.rearrange("(ko p) n -> p ko n", p=128)
    WnT_v = WnT.rearrange("(ko p) m -> p ko m", p=128)
    WsT_v = WsT.rearrange("(ko p) m -> p ko m", p=128)
    UnT_v = UnT.rearrange("(ko p) m -> p ko m", p=128)
    UfwT_v = UfwT.rearrange("(ko p) m -> p ko m", p=128)
    UsT_v = UsT.rearrange("(ko p) m -> p ko m", p=128)
    UfswT_v = UfswT.rearrange("(ko p) m -> p ko m", p=128)

    SIG = mybir.ActivationFunctionType.Sigmoid
    TANH = mybir.ActivationFunctionType.Tanh

    with tile.TileContext(nc) as tc, ExitStack() as stack:
        wp = stack.enter_context(tc.tile_pool(name="w", bufs=1))
        io = stack.enter_context(tc.tile_pool(name="io", bufs=2))
        mid = stack.enter_context(tc.tile_pool(name="mid", bufs=2))
        psp = stack.enter_context(tc.tile_pool(name="psp", bufs=4, space="PSUM"))

        # --- resident weights in need order: the two type-1 warmup singles
        # run first (Ufsw, Ws, Us), then type-0 (Ufw, Wn, Un); spread across
        # the three DMA rings so no single ring serializes the head ---
        Ufsw_sb = wp.tile([128, 2, H], FP16)
        nc.sync.dma_start(out=Ufsw_sb, in_=UfswT_v)
        bfs_sb = wp.tile([128, 2], F32)
        nc.sync.dma_start(out=bfs_sb, in_=bias_fs)
        bfn_sb = wp.tile([128, 4], F32)
        nc.gpsimd.dma_start(out=bfn_sb, in_=bias_fn)
        Ws_sb = wp.tile([128, 3, 3 * H], FP16)
        Us_sb = wp.tile([128, 2, 3 * H], FP16)
        nc.scalar.dma_start(out=Ws_sb[:, 0, :], in_=WsT_v[:, 0, :])
        nc.gpsimd.dma_start(out=Ws_sb[:, 1, :], in_=WsT_v[:, 1, :])
        nc.scalar.dma_start(out=Ws_sb[:, 2, :], in_=WsT_v[:, 2, :])
        nc.gpsimd.dma_start(out=Us_sb[:, 0, :], in_=UsT_v[:, 0, :])
        nc.scalar.dma_start(out=Us_sb[:, 1, :], in_=UsT_v[:, 1, :])
        # type-0 weights: allocated here, but their DMAs are emitted only
        # after the warmup singles' data loads so the ring queues serve the
        # head in true need order.
        Ufw_sb = wp.tile([128, 4, 2 * H], FP16)
        Wn_sb = wp.tile([128, 3, 3 * H], FP16)
        Un_sb = wp.tile([128, 4, 3 * H], FP16)
        br0_fired = [False]

        def load_br0_weights():
            if br0_fired[0]:
                return
            br0_fired[0] = True
            nc.sync.dma_start(out=Ufw_sb[:, 0:2, :], in_=UfwT_v[:, 0:2, :])
            nc.gpsimd.dma_start(out=Ufw_sb[:, 2:4, :], in_=UfwT_v[:, 2:4, :])
            nc.sync.dma_start(out=Wn_sb[:, 0, :], in_=WnT_v[:, 0, :])
            nc.scalar.dma_start(out=Wn_sb[:, 1, :], in_=WnT_v[:, 1, :])
            nc.gpsimd.dma_start(out=Wn_sb[:, 2, :], in_=WnT_v[:, 2, :])
            nc.sync.dma_start(out=Un_sb[:, 0, :], in_=UnT_v[:, 0, :])
            nc.scalar.dma_start(out=Un_sb[:, 1, :], in_=UnT_v[:, 1, :])
            nc.gpsimd.dma_start(out=Un_sb[:, 2, :], in_=UnT_v[:, 2, :])
            nc.sync.dma_start(out=Un_sb[:, 3, :], in_=UnT_v[:, 3, :])

        def do_block(br, xt, ht, ct, htild, j0, n0, jw, ncol=TILE_N):
            """Process jw (1 or 2) node tiles starting at macro tile j0.
            n0 = DRAM node offset; ncol<=512 nodes per tile (last tile of a
            segment may be a 256-node half tile)."""
            w = (jw - 1) * TILE_N + ncol
            cols = [slice((j0 + j) * TILE_N,
                          (j0 + j) * TILE_N + (TILE_N if j < jw - 1 else ncol))
                    for j in range(jw)]
            span = slice(j0 * TILE_N, j0 * TILE_N + w)

            f_full = mid.tile([128, 4, 2 * TILE_N], FP16, tag="f", name="f")
            f = f_full[:, :, :w]

            # --- forget gates ---
            if br == 0:
                for m in range(4):
                    ps = psp.tile([128, 2, TILE_N], F32, tag="ps", name="ps")
                    for j in range(jw):
                        nj = cols[j].stop - cols[j].start
                        for k in range(4):
                            nc.tensor.matmul(
                                ps[:, j, :nj],
                                Ufw_sb[:, k, 128 * m : 128 * (m + 1)],
                                ht[:, k, cols[j]],
                                start=(k == 0),
                                stop=(k == 3),
                            )
                    if ncol == TILE_N:
                        nc.scalar.activation(
                            out=f[:, m, :],
                            in_=ps[:, :jw, :].rearrange("p a b -> p (a b)"),
                            func=SIG,
                            bias=bfn_sb[:, m : m + 1],
                        )
                    else:
                        for j in range(jw):
                            nj = cols[j].stop - cols[j].start
                            nc.scalar.activation(
                                out=f[:, m, j * TILE_N : j * TILE_N + nj],
                                in_=ps[:, j, :nj],
                                func=SIG,
                                bias=bfn_sb[:, m : m + 1],
                            )
            else:
                for child in range(2):
                    for m in range(2):
                        ps = psp.tile([128, 2, TILE_N], F32, tag="ps", name="ps")
                        for j in range(jw):
                            nj = cols[j].stop - cols[j].start
                            for k in range(2):
                                nc.tensor.matmul(
                                    ps[:, j, :nj],
                                    Ufsw_sb[:, k, 128 * m : 128 * (m + 1)],
                                    ht[:, 2 * child + k, cols[j]],
                                    start=(k == 0),
                                    stop=(k == 1),
                                )
                        if ncol == TILE_N:
                            nc.scalar.activation(
                                out=f[:, 2 * child + m, :],
                                in_=ps[:, :jw, :].rearrange("p a b -> p (a b)"),
                                func=SIG,
                                bias=bfs_sb[:, m : m + 1],
                            )
                        else:
                            for j in range(jw):
                                nj = cols[j].stop - cols[j].start
                                nc.scalar.activation(
                                    out=f[:, 2 * child + m,
                                          j * TILE_N : j * TILE_N + nj],
                                    in_=ps[:, j, :nj],
                                    func=SIG,
                                    bias=bfs_sb[:, m : m + 1],
                                )

            # prod = f * c_child (in place); cred = child0 + child1
            nc.vector.tensor_mul(out=f, in0=f, in1=ct[:, :, span])
            cred_full = mid.tile([128, 2, 2 * TILE_N], F32, tag="cred", name="cred")
            cred = cred_full[:, :, :w]
            nc.vector.tensor_add(out=cred, in0=f[:, 0:2, :], in1=f[:, 2:4, :])

            # --- iou gates: 3 m-pair phases, weights shared across the
            # block's tiles ---
            gates_full = mid.tile([128, 6, 2 * TILE_N], FP16, tag="g", name="g")
            gates = gates_full[:, :, :w]
            for mp in range(3):
                pss = [
                    psp.tile([128, 2, TILE_N], F32, tag="ps", name="ps")
                    for _ in range(jw)
                ]
                for j in range(jw):
                    nj = cols[j].stop - cols[j].start
                    for m2 in range(2):
                        m = 2 * mp + m2
                        ms = slice(128 * m, 128 * (m + 1))
                        if br == 0:
                            for k in range(3):
                                nc.tensor.matmul(
                                    pss[j][:, m2, :nj], Wn_sb[:, k, ms],
                                    xt[:, k, cols[j]],
                                    start=(k == 0), stop=False,
                                )
                            for k in range(4):
                                nc.tensor.matmul(
                                    pss[j][:, m2, :nj], Un_sb[:, k, ms],
                                    ht[:, k, cols[j]],
                                    start=False, stop=(k == 3),
                                )
                        else:
                            for k in range(3):
                                nc.tensor.matmul(
                                    pss[j][:, m2, :nj], Ws_sb[:, k, ms],
                                    xt[:, k, cols[j]],
                                    start=(k == 0), stop=False,
                                )
                            for k in range(2):
                                nc.tensor.matmul(
                                    pss[j][:, m2, :nj], Us_sb[:, k, ms],
                                    htild[:, k, cols[j]],
                                    start=False, stop=(k == 1),
                                )
                for j in range(jw):
                    nj = cols[j].stop - cols[j].start
                    nc.scalar.activation(
                        out=gates[:, 2 * mp : 2 * mp + 2,
                                  j * TILE_N : j * TILE_N + nj],
                        in_=pss[j][:, :, :nj],
                        func=TANH if mp == 2 else SIG,
                    )

            # c = sig(i)*tanh(u) + cred (into gates[:,0:2])
            nc.vector.tensor_mul(
                out=gates[:, 0:2, :], in0=gates[:, 0:2, :], in1=gates[:, 4:6, :]
            )
            nc.vector.tensor_add(out=gates[:, 0:2, :], in0=gates[:, 0:2, :], in1=cred)
            nc.gpsimd.dma_start(out=cOT_v[:, :, n0 : n0 + w], in_=gates[:, 0:2, :])
            # h = sig(o)*tanh(c) (tanh into gates[:,4:6], h into gates[:,2:4])
            nc.scalar.activation(out=gates[:, 4:6, :], in_=gates[:, 0:2, :], func=TANH)
            nc.vector.tensor_mul(
                out=gates[:, 2:4, :], in0=gates[:, 2:4, :], in1=gates[:, 4:6, :]
            )
            nc.gpsimd.dma_start(out=hOT_v[:, :, n0 : n0 + w], in_=gates[:, 2:4, :])

        # --- macro schedule: two type-1 singles (cheap weight prefix
        # unblocks the PE early), then all type-0, then remaining type-1.
        # Counts are in 256-node halves; a segment's odd tail half-tile is
        # processed as a 256-node tile at the segment end. ---
        HALF = TILE_N // 2
        T0, half0 = divmod(T0h, 2)
        T1, half1 = divmod(T1h, 2)
        s1 = min(2, T1)
        base1 = T0h * HALF
        entries = []
        if s1:
            entries.append((1, base1, s1, half1, True))
        if T0 or half0:
            entries.append((0, 0, T0, half0, False))
        if (T1 - s1) or half1:
            entries.append((1, base1 + s1 * TILE_N, T1 - s1, half1, False))
        for ei, (br, base, T, half, singles) in enumerate(entries):
            starts = []
            g = 0
            while g < T:
                nt = 1 if singles else min(4, T - g)
                starts.append((g, nt))
                g += nt
            if half and not singles:
                # tack the 256-node tail tile onto the last macro (or its own)
                if starts and starts[-1][1] < 4:
                    starts[-1] = (starts[-1][0], starts[-1][1] + 0.5)
                else:
                    starts.append((g, 0.5))
            for gi, (g, ntf) in enumerate(starts):
                nt = int(ntf)
                tail = ntf != nt          # has a trailing 256-node half tile
                n0 = base + g * TILE_N
                w = nt * TILE_N + (HALF if tail else 0)
                nt_eff = nt + (1 if tail else 0)
                head = ei == 0 and gi == 0
                xt_full = io.tile([128, 3, MACRO], FP16, tag="xt", name="xt")
                xt = xt_full[:, :, :w]
                xeng = nc.gpsimd if head else nc.sync
                for k in range(3):
                    xeng.dma_start(out=xt[:, k, :], in_=xT_v[:, k, n0 : n0 + w])
                ht_full = io.tile([128, 4, MACRO], FP16, tag="ht", name="ht")
                ht = ht_full[:, :, :w]
                nc.sync.dma_start(out=ht, in_=hT_v[:, :, n0 : n0 + w])
                ct_full = io.tile([128, 4, MACRO], FP16, tag="ct", name="ct")
                ct = ct_full[:, :, :w]
                (nc.gpsimd if head else nc.scalar).dma_start(
                    out=ct, in_=cT_v[:, :, n0 : n0 + w]
                )
                if (ei == 0 and gi == len(starts) - 1) or br == 0:
                    load_br0_weights()

                htild = None
                if br == 1:
                    htild_full = io.tile(
                        [128, 2, MACRO], FP16, tag="htild", name="htild"
                    )
                    htild = htild_full[:, :, :w]
                    nc.vector.tensor_add(
                        out=htild, in0=ht[:, 0:2, :], in1=ht[:, 2:4, :]
                    )

                last = ei == len(entries) - 1 and gi == len(starts) - 1
                j0 = 0
                while j0 < nt_eff:
                    jw = 1 if (last and j0 >= nt_eff - 2) else min(2, nt_eff - j0)
                    ncol = HALF if (tail and j0 + jw == nt_eff) else TILE_N
                    do_block(br, xt, ht, ct, htild, j0, n0 + j0 * TILE_N, jw,
                             ncol)
                    j0 += jw

    nc.compile()
    _PROGRAM_CACHE[key] = nc
    return nc


def kernel(x, h_child, c_child, t, W_iou, U_iou, b_iou, U_f_w, U_f_b,
           W_iou_s, U_iou_s, b_iou_s, U_f_s_w, U_f_s_b):
    global LAST_EXEC_NS
    x = np.asarray(x, dtype=np.float32)
    h_child = np.asarray(h_child, dtype=np.float32)
    c_child = np.asarray(c_child, dtype=np.float32)
    t = np.asarray(t)
    n = x.shape[0]

    # --- host partition: equal per-core type counts, padded to tiles ---
    idx0 = np.flatnonzero(t == 0)
    idx1 = np.flatnonzero(t != 0)
    n0, n1 = len(idx0), len(idx1)

    def pad_split(idx, cnt):
        if cnt == 0:
            return np.zeros((CORES, 0), dtype=np.int64), 0
        per = _round_up(-(-cnt // CORES), TILE_N // 2)
        padded = np.concatenate(
            [idx, np.full(CORES * per - cnt, idx[-1], dtype=idx.dtype)]
        )
        return padded.reshape(CORES, per).astype(np.int64), per

    chunks0, P0 = pad_split(idx0, n0)
    chunks1, P1 = pad_split(idx1, n1)

    nc = _build_program(P0 // (TILE_N // 2), P1 // (TILE_N // 2))

    hc2 = h_child.reshape(n, 2 * H)
    cc2 = c_child.reshape(n, 2 * H)

    def bias_tile(v, m):
        return np.ascontiguousarray(
            np.asarray(v, np.float32).reshape(-1)[: 128 * m].reshape(m, 128).T
        )

    def w_with_bias(W, b):
        # [XP, 768] = W^T with bias as row 300, zero rows 301..383
        out = np.zeros((XP, 3 * H), dtype=NP_FP16)
        out[:X] = np.asarray(W, np.float32).T.astype(NP_FP16)
        out[X] = np.asarray(b, np.float32).reshape(-1).astype(NP_FP16)
        return out

    wmap = {
        "WnT": w_with_bias(W_iou, b_iou),
        "UnT": np.ascontiguousarray(np.asarray(U_iou, np.float32).T).astype(NP_FP16),
        "UfwT": np.ascontiguousarray(np.asarray(U_f_w, np.float32).T).astype(NP_FP16),
        "WsT": w_with_bias(W_iou_s, b_iou_s),
        "UsT": np.ascontiguousarray(np.asarray(U_iou_s, np.float32).T).astype(NP_FP16),
        "UfswT": np.ascontiguousarray(np.asarray(U_f_s_w, np.float32).T).astype(NP_FP16),
        "bias_fn": bias_tile(U_f_b, 4),
        "bias_fs": bias_tile(U_f_s_b, 2),
    }

    in_maps = []
    for i in range(CORES):
        I = np.concatenate([chunks0[i], chunks1[i]])
        xTi = np.zeros((XP, len(I)), dtype=NP_FP16)
        xTi[:X] = x[I].T.astype(NP_FP16)
        xTi[X] = 1.0
        m = dict(wmap)
        m["xT"] = xTi
        m["hT"] = hc2[I].T.astype(NP_FP16)
        m["cT"] = cc2[I].T.astype(NP_FP16)
        in_maps.append(m)

    res = bass_utils.run_bass_kernel_spmd(
        nc, in_maps, core_ids=list(range(CORES)), trace=TRACE
    )
    LAST_EXEC_NS = res.exec_time_ns

    # --- scatter back ---
    h_out = np.empty((n, H), dtype=np.float32)
    c_out = np.empty((n, H), dtype=np.float32)
    if n0:
        h0 = np.concatenate(
            [res.results[i]["hOT"][:, :P0].T.astype(np.float32) for i in range(CORES)]
        )
        c0 = np.concatenate(
            [res.results[i]["cOT"][:, :P0].T.astype(np.float32) for i in range(CORES)]
        )
        h_out[idx0] = h0[:n0]
        c_out[idx0] = c0[:n0]
    if n1:
        h1 = np.concatenate(
            [res.results[i]["hOT"][:, P0:].T.astype(np.float32) for i in range(CORES)]
        )
        c1 = np.concatenate(
            [res.results[i]["cOT"][:, P0:].T.astype(np.float32) for i in range(CORES)]
        )
        h_out[idx1] = h1[:n1]
        c_out[idx1] = c1[:n1]
    return h_out, c_out


# revision 8
# speedup vs baseline: 1.2025x; 1.0023x over previous
"""MixTreeLSTMCell Trainium2 kernel (8 NeuronCores, SPMD).

The cell evaluates one of two branches per node (t in {0,1}); the host
partitions nodes by type so each core runs two static branch segments with
no per-node select, on feature-major fp16 operands. The device program is
tensor-engine-bound (~1574 fp16 matmuls/core at ~218ns), so the design
keeps the PE gap-free:
- x^T zero-padded on host from 301 to 384 contraction rows: every matmul
  is a full 128-partition instruction (sub-128-row matmuls measured 1.5x
  slower per moving row on HW).
- Node tiles processed in blocks of two 512-node tiles with bank-stable
  PSUM targeting (runs of consecutive matmuls into one bank), rotating
  through one 8-bank PSUM pool deep enough that ACT drains never stall
  the PE.
- Macro schedule: two type-1 single-tile warmups first (their small
  f/iou weights load fastest, unblocking the PE at ~10us), then all
  type-0 macros, then the remaining type-1; type-0 weights stream in
  lazily behind the warmups' data in ring-queue need order.
- Per-core type counts padded to 256-node halves (segment tails run one
  256-wide tile) to minimize padded compute.
- fp16 outputs; iou bias folded into the matmul via a ones row; f-gate
  biases applied by the scalar engine's free bias; the elementwise chain
  runs in place inside the gates tile and h/c are DMA'd straight out of
  it on the SWDGE ring.
"""

from contextlib import ExitStack

import numpy as np

import concourse.bacc as bacc
import concourse.tile as tile
from concourse import mybir
from concourse import bass_utils

F32 = mybir.dt.float32
FP16 = mybir.dt.float16
NP_FP16 = np.float16

N_NODES = 131072
X = 300
XP = 384              # x rows + ones(bias) row at 300, zero-padded to 3*128
H = 256
CORES = 8
TILE_N = 512          # nodes per matmul tile (max moving free dim)
MACRO = 4 * TILE_N    # nodes per DMA macro tile

TRACE = False
LAST_EXEC_NS = None

_PROGRAM_CACHE = {}


def _round_up(v, m):
    return (v + m - 1) // m * m


def _build_program(T0h, T1h):
    """T0h/T1h: per-core type-0/1 node counts in 256-node half-tile units."""
    key = (T0h, T1h)
    if key in _PROGRAM_CACHE:
        return _PROGRAM_CACHE[key]

    Nc = (T0h + T1h) * (TILE_N // 2)
    nc = bacc.Bacc("TRN2", target_bir_lowering=False, debug=False)

    xT = nc.dram_tensor("xT", [XP, Nc], FP16, kind="ExternalInput").ap()
    hT = nc.dram_tensor("hT", [2 * H, Nc], FP16, kind="ExternalInput").ap()
    cT = nc.dram_tensor("cT", [2 * H, Nc], FP16, kind="ExternalInput").ap()

    WnT = nc.dram_tensor("WnT", [XP, 3 * H], FP16, kind="ExternalInput").ap()
    UnT = nc.dram_tensor("UnT", [2 * H, 3 * H], FP16, kind="ExternalInput").ap()
    UfwT = nc.dram_tensor("UfwT", [2 * H, 2 * H], FP16, kind="ExternalInput").ap()
    WsT = nc.dram_tensor("WsT", [XP, 3 * H], FP16, kind="ExternalInput").ap()
    UsT = nc.dram_tensor("UsT", [H, 3 * H], FP16, kind="ExternalInput").ap()
    UfswT = nc.dram_tensor("UfswT", [H, H], FP16, kind="ExternalInput").ap()

    bias_fn = nc.dram_tensor("bias_fn", [128, 4], F32, kind="ExternalInput").ap()
    bias_fs = nc.dram_tensor("bias_fs", [128, 2], F32, kind="ExternalInput").ap()

    hOT = nc.dram_tensor("hOT", [H, Nc], FP16, kind="ExternalOutput").ap()
    cOT = nc.dram_tensor("cOT", [H, Nc], FP16, kind="ExternalOutput").ap()

    xT_v = xT.rearrange("(ko p) n -> p ko n", p=128)
    hT_v = hT.rearrange("(ko p) n -> p ko n", p=128)
    cT_v = cT.rearrange("(ko p) n -> p ko n", p=128)
    hOT_v = hOT.rearrange("(ko p) n -> p ko n", p=128)
    cOT_v = cOT.rearrange("(ko p) n -> p ko n", p=128)
    WnT_v = WnT.rearrange("(ko p) m -> p ko m", p=128)
    WsT_v = WsT.rearrange("(ko p) m -> p ko m", p=128)
    UnT_v = UnT.rearrange("(ko p) m -> p ko m", p=128)
    UfwT_v = UfwT.rearrange("(ko p) m -> p ko m", p=128)
    UsT_v = UsT.rearrange("(ko p) m -> p ko m", p=128)
    UfswT_v = UfswT.rearrange("(ko p) m -> p ko m", p=128)

    SIG = mybir.ActivationFunctionType.Sigmoid
    TANH = mybir.ActivationFunctionType.Tanh

    with tile.TileContext(nc) as tc, ExitStack() as stack:
        wp = stack.enter_context(tc.tile_pool(name="w", bufs=1))
        io = stack.enter_context(tc.tile_pool(name="io", bufs=2))
        mid = stack.enter_context(tc.tile_pool(name="mid", bufs=2))
        psp = stack.enter_context(tc.tile_pool(name="psp", bufs=4, space="PSUM"))

        # --- resident weights in need order: the two type-1 warmup singles
        # run first (Ufsw, Ws, Us), then type-0 (Ufw, Wn, Un); spread across
        # the three DMA rings so no single ring serializes the head ---
        Ufsw_sb = wp.tile([128, 2, H], FP16)
        nc.sync.dma_start(out=Ufsw_sb, in_=UfswT_v)
        bfs_sb = wp.tile([128, 2], F32)
        nc.sync.dma_start(out=bfs_sb, in_=bias_fs)
        bfn_sb = wp.tile([128, 4], F32)
        nc.gpsimd.dma_start(out=bfn_sb, in_=bias_fn)
        Ws_sb = wp.tile([128, 3, 3 * H], FP16)
        Us_sb = wp.tile([128, 2, 3 * H], FP16)
        nc.scalar.dma_start(out=Ws_sb[:, 0, :], in_=WsT_v[:, 0, :])
        nc.scalar.dma_start(out=Ws_sb[:, 1, :], in_=WsT_v[:, 1, :])
        nc.scalar.dma_start(out=Ws_sb[:, 2, :], in_=WsT_v[:, 2, :])
        nc.scalar.dma_start(out=Us_sb[:, 0, :], in_=UsT_v[:, 0, :])
        nc.scalar.dma_start(out=Us_sb[:, 1, :], in_=UsT_v[:, 1, :])
        # type-0 weights: allocated here, but their DMAs are emitted only
        # after the warmup singles' data loads so the ring queues serve the
        # head in true need order.
        Ufw_sb = wp.tile([128, 4, 2 * H], FP16)
        Wn_sb = wp.tile([128, 3, 3 * H], FP16)
        Un_sb = wp.tile([128, 4, 3 * H], FP16)
        br0_fired = [False]

        def load_br0_weights():
            if br0_fired[0]:
                return
            br0_fired[0] = True
            nc.sync.dma_start(out=Ufw_sb[:, 0:2, :], in_=UfwT_v[:, 0:2, :])
            nc.gpsimd.dma_start(out=Ufw_sb[:, 2:4, :], in_=UfwT_v[:, 2:4, :])
            nc.sync.dma_start(out=Wn_sb[:, 0, :], in_=WnT_v[:, 0, :])
            nc.scalar.dma_start(out=Wn_sb[:, 1, :], in_=WnT_v[:, 1, :])
            nc.gpsimd.dma_start(out=Wn_sb[:, 2, :], in_=WnT_v[:, 2, :])
            nc.sync.dma_start(out=Un_sb[:, 0, :], in_=UnT_v[:, 0, :])
            nc.scalar.dma_start(out=Un_sb[:, 1, :], in_=UnT_v[:, 1, :])
            nc.gpsimd.dma_start(out=Un_sb[:, 2, :], in_=UnT_v[:, 2, :])
            nc.sync.dma_start(out=Un_sb[:, 3, :], in_=UnT_v[:, 3, :])

        def do_block(br, xt, ht, ct, htild, j0, n0, jw, ncol=TILE_N):
            """Process jw (1 or 2) node tiles starting at macro tile j0.
            n0 = DRAM node offset; ncol<=512 nodes per tile (last tile of a
            segment may be a 256-node half tile)."""
            w = (jw - 1) * TILE_N + ncol
            cols = [slice((j0 + j) * TILE_N,
                          (j0 + j) * TILE_N + (TILE_N if j < jw - 1 else ncol))
                    for j in range(jw)]
            span = slice(j0 * TILE_N, j0 * TILE_N + w)

            f_full = mid.tile([128, 4, 2 * TILE_N], FP16, tag="f", name="f")
            f = f_full[:, :, :w]

            # --- forget gates ---
            if br == 0:
                for m in range(4):
                    ps = psp.tile([128, 2, TILE_N], F32, tag="ps", name="ps")
                    for j in range(jw):
                        nj = cols[j].stop - cols[j].start
                        for k in range(4):
                            nc.tensor.matmul(
                                ps[:, j, :nj],
                                Ufw_sb[:, k, 128 * m : 128 * (m + 1)],
                                ht[:, k, cols[j]],
                                start=(k == 0),
                                stop=(k == 3),
                            )
                    if ncol == TILE_N:
                        nc.scalar.activation(
                            out=f[:, m, :],
                            in_=ps[:, :jw, :].rearrange("p a b -> p (a b)"),
                            func=SIG,
                            bias=bfn_sb[:, m : m + 1],
                        )
                    else:
                        for j in range(jw):
                            nj = cols[j].stop - cols[j].start
                            nc.scalar.activation(
                                out=f[:, m, j * TILE_N : j * TILE_N + nj],
                                in_=ps[:, j, :nj],
                                func=SIG,
                                bias=bfn_sb[:, m : m + 1],
                            )
            else:
                for child in range(2):
                    for m in range(2):
                        ps = psp.tile([128, 2, TILE_N], F32, tag="ps", name="ps")
                        for j in range(jw):
                            nj = cols[j].stop - cols[j].start
                            for k in range(2):
                                nc.tensor.matmul(
                                    ps[:, j, :nj],
                                    Ufsw_sb[:, k, 128 * m : 128 * (m + 1)],
                                    ht[:, 2 * child + k, cols[j]],
                                    start=(k == 0),
                                    stop=(k == 1),
                                )
                        if ncol == TILE_N:
                            nc.scalar.activation(
                                out=f[:, 2 * child + m, :],
                                in_=ps[:, :jw, :].rearrange("p a b -> p (a b)"),
                                func=SIG,
                                bias=bfs_sb[:, m : m + 1],
                            )
                        else:
                            for j in range(jw):
                                nj = cols[j].stop - cols[j].start
                                nc.scalar.activation(
                                    out=f[:, 2 * child + m,
                                          j * TILE_N : j * TILE_N + nj],
                                    in_=ps[:, j, :nj],
                                    func=SIG,
                                    bias=bfs_sb[:, m : m + 1],
                                )

            # prod = f * c_child (in place); cred = child0 + child1
            nc.vector.tensor_mul(out=f, in0=f, in1=ct[:, :, span])
            cred_full = mid.tile([128, 2, 2 * TILE_N], F32, tag="cred", name="cred")
            cred = cred_full[:, :, :w]
            nc.vector.tensor_add(out=cred, in0=f[:, 0:2, :], in1=f[:, 2:4, :])

            # --- iou gates: 3 m-pair phases, weights shared across the
            # block's tiles ---
            gates_full = mid.tile([128, 6, 2 * TILE_N], FP16, tag="g", name="g")
            gates = gates_full[:, :, :w]
            for mp in range(3):
                pss = [
                    psp.tile([128, 2, TILE_N], F32, tag="ps", name="ps")
                    for _ in range(jw)
                ]
                for j in range(jw):
                    nj = cols[j].stop - cols[j].start
                    for m2 in range(2):
                        m = 2 * mp + m2
                        ms = slice(128 * m, 128 * (m + 1))
                        if br == 0:
                            for k in range(3):
                                nc.tensor.matmul(
                                    pss[j][:, m2, :nj], Wn_sb[:, k, ms],
                                    xt[:, k, cols[j]],
                                    start=(k == 0), stop=False,
                                )
                            for k in range(4):
                                nc.tensor.matmul(
                                    pss[j][:, m2, :nj], Un_sb[:, k, ms],
                                    ht[:, k, cols[j]],
                                    start=False, stop=(k == 3),
                                )
                        else:
                            for k in range(3):
                                nc.tensor.matmul(
                                    pss[j][:, m2, :nj], Ws_sb[:, k, ms],
                                    xt[:, k, cols[j]],
                                    start=(k == 0), stop=False,
                                )
                            for k in range(2):
                                nc.tensor.matmul(
                                    pss[j][:, m2, :nj], Us_sb[:, k, ms],
                                    htild[:, k, cols[j]],
                                    start=False, stop=(k == 1),
                                )
                for j in range(jw):
                    nj = cols[j].stop - cols[j].start
                    nc.scalar.activation(
                        out=gates[:, 2 * mp : 2 * mp + 2,
                                  j * TILE_N : j * TILE_N + nj],
                        in_=pss[j][:, :, :nj],
                        func=TANH if mp == 2 else SIG,
                    )

            # c = sig(i)*tanh(u) + cred (into gates[:,0:2])
            nc.vector.tensor_mul(
                out=gates[:, 0:2, :], in0=gates[:, 0:2, :], in1=gates[:, 4:6, :]
            )
            nc.vector.tensor_add(out=gates[:, 0:2, :], in0=gates[:, 0:2, :], in1=cred)
            nc.gpsimd.dma_start(out=cOT_v[:, :, n0 : n0 + w], in_=gates[:, 0:2, :])
            # h = sig(o)*tanh(c) (tanh into gates[:,4:6], h into gates[:,2:4])
            nc.scalar.activation(out=gates[:, 4:6, :], in_=gates[:, 0:2, :], func=TANH)
            nc.vector.tensor_mul(
                out=gates[:, 2:4, :], in0=gates[:, 2:4, :], in1=gates[:, 4:6, :]
            )
            nc.gpsimd.dma_start(out=hOT_v[:, :, n0 : n0 + w], in_=gates[:, 2:4, :])

        # --- macro schedule: two type-1 singles (cheap weight prefix
        # unblocks the PE early), then all type-0, then remaining type-1.
        # Counts are in 256-node halves; a segment's odd tail half-tile is
        # processed as a 256-node tile at the segment end. ---
        HALF = TILE_N // 2
        T0, half0 = divmod(T0h, 2)
        T1, half1 = divmod(T1h, 2)
        s1 = min(3, T1)
        base1 = T0h * HALF
        entries = []
        if s1:
            entries.append((1, base1, s1, half1, True))
        if T0 or half0:
            entries.append((0, 0, T0, half0, False))
        if (T1 - s1) or half1:
            entries.append((1, base1 + s1 * TILE_N, T1 - s1, half1, False))
        for ei, (br, base, T, half, singles) in enumerate(entries):
            starts = []
            g = 0
            while g < T:
                nt = 1 if singles else min(4, T - g)
                starts.append((g, nt))
                g += nt
            if half and not singles:
                # tack the 256-node tail tile onto the last macro (or its own)
                if starts and starts[-1][1] < 4:
                    starts[-1] = (starts[-1][0], starts[-1][1] + 0.5)
                else:
                    starts.append((g, 0.5))
            for gi, (g, ntf) in enumerate(starts):
                nt = int(ntf)
                tail = ntf != nt          # has a trailing 256-node half tile
                n0 = base + g * TILE_N
                w = nt * TILE_N + (HALF if tail else 0)
                nt_eff = nt + (1 if tail else 0)
                head = ei == 0 and gi == 0
                xt_full = io.tile([128, 3, MACRO], FP16, tag="xt", name="xt")
                xt = xt_full[:, :, :w]
                ht_full = io.tile([128, 4, MACRO], FP16, tag="ht", name="ht")
                ht = ht_full[:, :, :w]
                ct_full = io.tile([128, 4, MACRO], FP16, tag="ct", name="ct")
                ct = ct_full[:, :, :w]
                if head:
                    # f-gates consume ht k0..k3 first, then iou needs xt;
                    # ct is only read by the vector engine later.
                    nc.gpsimd.dma_start(
                        out=ht[:, 2:4, :], in_=hT_v[:, 2:4, n0 : n0 + w]
                    )
                    for k in range(3):
                        nc.gpsimd.dma_start(
                            out=xt[:, k, :], in_=xT_v[:, k, n0 : n0 + w]
                        )
                    nc.sync.dma_start(
                        out=ht[:, 0:2, :], in_=hT_v[:, 0:2, n0 : n0 + w]
                    )
                    nc.sync.dma_start(
                        out=ct[:, 0:2, :], in_=cT_v[:, 0:2, n0 : n0 + w]
                    )
                    nc.scalar.dma_start(
                        out=ct[:, 2:4, :], in_=cT_v[:, 2:4, n0 : n0 + w]
                    )
                else:
                    for k in range(3):
                        nc.sync.dma_start(
                            out=xt[:, k, :], in_=xT_v[:, k, n0 : n0 + w]
                        )
                    nc.sync.dma_start(out=ht, in_=hT_v[:, :, n0 : n0 + w])
                    nc.scalar.dma_start(out=ct, in_=cT_v[:, :, n0 : n0 + w])
                if (ei == 0 and gi == len(starts) - 1) or br == 0:
                    load_br0_weights()

                htild = None
                if br == 1:
                    htild_full = io.tile(
                        [128, 2, MACRO], FP16, tag="htild", name="htild"
                    )
                    htild = htild_full[:, :, :w]
                    nc.vector.tensor_add(
                        out=htild, in0=ht[:, 0:2, :], in1=ht[:, 2:4, :]
                    )

                last = ei == len(entries) - 1 and gi == len(starts) - 1
                j0 = 0
                while j0 < nt_eff:
                    jw = 1 if (last and j0 >= nt_eff - 2) else min(2, nt_eff - j0)
                    ncol = HALF if (tail and j0 + jw == nt_eff) else TILE_N
                    do_block(br, xt, ht, ct, htild, j0, n0 + j0 * TILE_N, jw,
                             ncol)
                    j0 += jw

    nc.compile()
    _PROGRAM_CACHE[key] = nc
    return nc


def kernel(x, h_child, c_child, t, W_iou, U_iou, b_iou, U_f_w, U_f_b,
           W_iou_s, U_iou_s, b_iou_s, U_f_s_w, U_f_s_b):
    global LAST_EXEC_NS
    x = np.asarray(x, dtype=np.float32)
    h_child = np.asarray(h_child, dtype=np.float32)
    c_child = np.asarray(c_child, dtype=np.float32)
    t = np.asarray(t)
    n = x.shape[0]

    # --- host partition: equal per-core type counts, padded to tiles ---
    idx0 = np.flatnonzero(t == 0)
    idx1 = np.flatnonzero(t != 0)
    n0, n1 = len(idx0), len(idx1)

    def pad_split(idx, cnt):
        if cnt == 0:
            return np.zeros((CORES, 0), dtype=np.int64), 0
        per = _round_up(-(-cnt // CORES), TILE_N // 2)
        padded = np.concatenate(
            [idx, np.full(CORES * per - cnt, idx[-1], dtype=idx.dtype)]
        )
        return padded.reshape(CORES, per).astype(np.int64), per

    chunks0, P0 = pad_split(idx0, n0)
    chunks1, P1 = pad_split(idx1, n1)

    nc = _build_program(P0 // (TILE_N // 2), P1 // (TILE_N // 2))

    hc2 = h_child.reshape(n, 2 * H)
    cc2 = c_child.reshape(n, 2 * H)

    def bias_tile(v, m):
        return np.ascontiguousarray(
            np.asarray(v, np.float32).reshape(-1)[: 128 * m].reshape(m, 128).T
        )

    def w_with_bias(W, b):
        # [XP, 768] = W^T with bias as row 300, zero rows 301..383
        out = np.zeros((XP, 3 * H), dtype=NP_FP16)
        out[:X] = np.asarray(W, np.float32).T.astype(NP_FP16)
        out[X] = np.asarray(b, np.float32).reshape(-1).astype(NP_FP16)
        return out

    wmap = {
        "WnT": w_with_bias(W_iou, b_iou),
        "UnT": np.ascontiguousarray(np.asarray(U_iou, np.float32).T).astype(NP_FP16),
        "UfwT": np.ascontiguousarray(np.asarray(U_f_w, np.float32).T).astype(NP_FP16),
        "WsT": w_with_bias(W_iou_s, b_iou_s),
        "UsT": np.ascontiguousarray(np.asarray(U_iou_s, np.float32).T).astype(NP_FP16),
        "UfswT": np.ascontiguousarray(np.asarray(U_f_s_w, np.float32).T).astype(NP_FP16),
        "bias_fn": bias_tile(U_f_b, 4),
        "bias_fs": bias_tile(U_f_s_b, 2),
    }

    in_maps = []
    for i in range(CORES):
        I = np.concatenate([chunks0[i], chunks1[i]])
        xTi = np.zeros((XP, len(I)), dtype=NP_FP16)
        xTi[:X] = x[I].T.astype(NP_FP16)
        xTi[X] = 1.0
        m = dict(wmap)
        m["xT"] = xTi
        m["hT"] = hc2[I].T.astype(NP_FP16)
        m["cT"] = cc2[I].T.astype(NP_FP16)
        in_maps.append(m)

    res = bass_utils.run_bass_kernel_spmd(
        nc, in_maps, core_ids=list(range(CORES)), trace=TRACE
    )
    LAST_EXEC_NS = res.exec_time_ns

    # --- scatter back ---
    h_out = np.empty((n, H), dtype=np.float32)
    c_out = np.empty((n, H), dtype=np.float32)
    if n0:
        h0 = np.concatenate(
            [res.results[i]["hOT"][:, :P0].T.astype(np.float32) for i in range(CORES)]
        )
        c0 = np.concatenate(
            [res.results[i]["cOT"][:, :P0].T.astype(np.float32) for i in range(CORES)]
        )
        h_out[idx0] = h0[:n0]
        c_out[idx0] = c0[:n0]
    if n1:
        h1 = np.concatenate(
            [res.results[i]["hOT"][:, P0:].T.astype(np.float32) for i in range(CORES)]
        )
        c1 = np.concatenate(
            [res.results[i]["cOT"][:, P0:].T.astype(np.float32) for i in range(CORES)]
        )
        h_out[idx1] = h1[:n1]
        c_out[idx1] = c1[:n1]
    return h_out, c_out


# revision 9
# speedup vs baseline: 1.2040x; 1.0013x over previous
"""MixTreeLSTMCell Trainium2 kernel (8 NeuronCores, SPMD).

The cell evaluates one of two branches per node (t in {0,1}); the host
partitions nodes by type so each core runs two static branch segments with
no per-node select, on feature-major fp16 operands. The device program is
tensor-engine-bound (~1574 fp16 matmuls/core at ~218ns), so the design
keeps the PE gap-free:
- x^T zero-padded on host from 301 to 384 contraction rows: every matmul
  is a full 128-partition instruction (sub-128-row matmuls measured 1.5x
  slower per moving row on HW).
- Node tiles processed in blocks of two 512-node tiles with bank-stable
  PSUM targeting (runs of consecutive matmuls into one bank), rotating
  through one 8-bank PSUM pool deep enough that ACT drains never stall
  the PE.
- Macro schedule: two type-1 single-tile warmups first (their small
  f/iou weights load fastest, unblocking the PE at ~10us), then all
  type-0 macros, then the remaining type-1; type-0 weights stream in
  lazily behind the warmups' data in ring-queue need order.
- Per-core type counts padded to 256-node halves (segment tails run one
  256-wide tile) to minimize padded compute.
- fp16 outputs; iou bias folded into the matmul via a ones row; f-gate
  biases applied by the scalar engine's free bias; the elementwise chain
  runs in place inside the gates tile and h/c are DMA'd straight out of
  it on the SWDGE ring.
"""

from contextlib import ExitStack

import numpy as np

import concourse.bacc as bacc
import concourse.tile as tile
from concourse import mybir
from concourse import bass_utils

F32 = mybir.dt.float32
FP16 = mybir.dt.float16
NP_FP16 = np.float16

N_NODES = 131072
X = 300
XP = 384              # x rows + ones(bias) row at 300, zero-padded to 3*128
H = 256
CORES = 8
TILE_N = 512          # nodes per matmul tile (max moving free dim)
MACRO = 4 * TILE_N    # nodes per DMA macro tile

TRACE = False
LAST_EXEC_NS = None

_PROGRAM_CACHE = {}


def _round_up(v, m):
    return (v + m - 1) // m * m


def _build_program(T0h, T1h):
    """T0h/T1h: per-core type-0/1 node counts in 256-node half-tile units."""
    key = (T0h, T1h)
    if key in _PROGRAM_CACHE:
        return _PROGRAM_CACHE[key]

    Nc = (T0h + T1h) * (TILE_N // 2)
    nc = bacc.Bacc("TRN2", target_bir_lowering=False, debug=False)

    xT = nc.dram_tensor("xT", [XP, Nc], FP16, kind="ExternalInput").ap()
    hT = nc.dram_tensor("hT", [2 * H, Nc], FP16, kind="ExternalInput").ap()
    cT = nc.dram_tensor("cT", [2 * H, Nc], FP16, kind="ExternalInput").ap()

    WnT = nc.dram_tensor("WnT", [XP, 3 * H], FP16, kind="ExternalInput").ap()
    UnT = nc.dram_tensor("UnT", [2 * H, 3 * H], FP16, kind="ExternalInput").ap()
    UfwT = nc.dram_tensor("UfwT", [2 * H, 2 * H], FP16, kind="ExternalInput").ap()
    WsT = nc.dram_tensor("WsT", [XP, 3 * H], FP16, kind="ExternalInput").ap()
    UsT = nc.dram_tensor("UsT", [H, 3 * H], FP16, kind="ExternalInput").ap()
    UfswT = nc.dram_tensor("UfswT", [H, H], FP16, kind="ExternalInput").ap()

    bias_fn = nc.dram_tensor("bias_fn", [128, 4], F32, kind="ExternalInput").ap()
    bias_fs = nc.dram_tensor("bias_fs", [128, 2], F32, kind="ExternalInput").ap()

    hOT = nc.dram_tensor("hOT", [H, Nc], FP16, kind="ExternalOutput").ap()
    cOT = nc.dram_tensor("cOT", [H, Nc], FP16, kind="ExternalOutput").ap()

    xT_v = xT.rearrange("(ko p) n -> p ko n", p=128)
    hT_v = hT.rearrange("(ko p) n -> p ko n", p=128)
    cT_v = cT.rearrange("(ko p) n -> p ko n", p=128)
    hOT_v = hOT.rearrange("(ko p) n -> p ko n", p=128)
    cOT_v = cOT.rearrange("(ko p) n -> p ko n", p=128)
    WnT_v = WnT.rearrange("(ko p) m -> p ko m", p=128)
    WsT_v = WsT.rearrange("(ko p) m -> p ko m", p=128)
    UnT_v = UnT.rearrange("(ko p) m -> p ko m", p=128)
    UfwT_v = UfwT.rearrange("(ko p) m -> p ko m", p=128)
    UsT_v = UsT.rearrange("(ko p) m -> p ko m", p=128)
    UfswT_v = UfswT.rearrange("(ko p) m -> p ko m", p=128)

    SIG = mybir.ActivationFunctionType.Sigmoid
    TANH = mybir.ActivationFunctionType.Tanh

    with tile.TileContext(nc) as tc, ExitStack() as stack:
        wp = stack.enter_context(tc.tile_pool(name="w", bufs=1))
        io = stack.enter_context(tc.tile_pool(name="io", bufs=2))
        mid = stack.enter_context(tc.tile_pool(name="mid", bufs=2))
        psp = stack.enter_context(tc.tile_pool(name="psp", bufs=4, space="PSUM"))

        # --- resident weights in need order: the two type-1 warmup singles
        # run first (Ufsw, Ws, Us), then type-0 (Ufw, Wn, Un); spread across
        # the three DMA rings so no single ring serializes the head ---
        Ufsw_sb = wp.tile([128, 2, H], FP16)
        nc.sync.dma_start(out=Ufsw_sb, in_=UfswT_v)
        bfs_sb = wp.tile([128, 2], F32)
        nc.sync.dma_start(out=bfs_sb, in_=bias_fs)
        bfn_sb = wp.tile([128, 4], F32)
        nc.gpsimd.dma_start(out=bfn_sb, in_=bias_fn)
        Ws_sb = wp.tile([128, 3, 3 * H], FP16)
        Us_sb = wp.tile([128, 2, 3 * H], FP16)
        nc.scalar.dma_start(out=Ws_sb[:, 0, :], in_=WsT_v[:, 0, :])
        nc.gpsimd.dma_start(out=Ws_sb[:, 1, :], in_=WsT_v[:, 1, :])
        nc.scalar.dma_start(out=Ws_sb[:, 2, :], in_=WsT_v[:, 2, :])
        nc.gpsimd.dma_start(out=Us_sb[:, 0, :], in_=UsT_v[:, 0, :])
        nc.scalar.dma_start(out=Us_sb[:, 1, :], in_=UsT_v[:, 1, :])
        # type-0 weights: allocated here, but their DMAs are emitted only
        # after the warmup singles' data loads so the ring queues serve the
        # head in true need order.
        Ufw_sb = wp.tile([128, 4, 2 * H], FP16)
        Wn_sb = wp.tile([128, 3, 3 * H], FP16)
        Un_sb = wp.tile([128, 4, 3 * H], FP16)
        br0_fired = [False]

        def load_br0_weights():
            if br0_fired[0]:
                return
            br0_fired[0] = True
            nc.sync.dma_start(out=Ufw_sb[:, 0:2, :], in_=UfwT_v[:, 0:2, :])
            nc.gpsimd.dma_start(out=Ufw_sb[:, 2:4, :], in_=UfwT_v[:, 2:4, :])
            nc.sync.dma_start(out=Wn_sb[:, 0, :], in_=WnT_v[:, 0, :])
            nc.scalar.dma_start(out=Wn_sb[:, 1, :], in_=WnT_v[:, 1, :])
            nc.gpsimd.dma_start(out=Wn_sb[:, 2, :], in_=WnT_v[:, 2, :])
            nc.sync.dma_start(out=Un_sb[:, 0, :], in_=UnT_v[:, 0, :])
            nc.scalar.dma_start(out=Un_sb[:, 1, :], in_=UnT_v[:, 1, :])
            nc.gpsimd.dma_start(out=Un_sb[:, 2, :], in_=UnT_v[:, 2, :])
            nc.sync.dma_start(out=Un_sb[:, 3, :], in_=UnT_v[:, 3, :])

        def do_block(br, xt, ht, ct, htild, j0, n0, jw, ncol=TILE_N):
            """Process jw (1 or 2) node tiles starting at macro tile j0.
            n0 = DRAM node offset; ncol<=512 nodes per tile (last tile of a
            segment may be a 256-node half tile)."""
            w = (jw - 1) * TILE_N + ncol
            cols = [slice((j0 + j) * TILE_N,
                          (j0 + j) * TILE_N + (TILE_N if j < jw - 1 else ncol))
                    for j in range(jw)]
            span = slice(j0 * TILE_N, j0 * TILE_N + w)

            f_full = mid.tile([128, 4, 2 * TILE_N], FP16, tag="f", name="f")
            f = f_full[:, :, :w]

            # --- forget gates ---
            if br == 0:
                for m in range(4):
                    ps = psp.tile([128, 2, TILE_N], F32, tag="ps", name="ps")
                    for j in range(jw):
                        nj = cols[j].stop - cols[j].start
                        for k in range(4):
                            nc.tensor.matmul(
                                ps[:, j, :nj],
                                Ufw_sb[:, k, 128 * m : 128 * (m + 1)],
                                ht[:, k, cols[j]],
                                start=(k == 0),
                                stop=(k == 3),
                            )
                    if ncol == TILE_N:
                        nc.scalar.activation(
                            out=f[:, m, :],
                            in_=ps[:, :jw, :].rearrange("p a b -> p (a b)"),
                            func=SIG,
                            bias=bfn_sb[:, m : m + 1],
                        )
                    else:
                        for j in range(jw):
                            nj = cols[j].stop - cols[j].start
                            nc.scalar.activation(
                                out=f[:, m, j * TILE_N : j * TILE_N + nj],
                                in_=ps[:, j, :nj],
                                func=SIG,
                                bias=bfn_sb[:, m : m + 1],
                            )
            else:
                for child in range(2):
                    for m in range(2):
                        ps = psp.tile([128, 2, TILE_N], F32, tag="ps", name="ps")
                        for j in range(jw):
                            nj = cols[j].stop - cols[j].start
                            for k in range(2):
                                nc.tensor.matmul(
                                    ps[:, j, :nj],
                                    Ufsw_sb[:, k, 128 * m : 128 * (m + 1)],
                                    ht[:, 2 * child + k, cols[j]],
                                    start=(k == 0),
                                    stop=(k == 1),
                                )
                        if ncol == TILE_N:
                            nc.scalar.activation(
                                out=f[:, 2 * child + m, :],
                                in_=ps[:, :jw, :].rearrange("p a b -> p (a b)"),
                                func=SIG,
                                bias=bfs_sb[:, m : m + 1],
                            )
                        else:
                            for j in range(jw):
                                nj = cols[j].stop - cols[j].start
                                nc.scalar.activation(
                                    out=f[:, 2 * child + m,
                                          j * TILE_N : j * TILE_N + nj],
                                    in_=ps[:, j, :nj],
                                    func=SIG,
                                    bias=bfs_sb[:, m : m + 1],
                                )

            # prod = f * c_child (in place); cred = child0 + child1
            nc.vector.tensor_mul(out=f, in0=f, in1=ct[:, :, span])
            cred_full = mid.tile([128, 2, 2 * TILE_N], F32, tag="cred", name="cred")
            cred = cred_full[:, :, :w]
            nc.vector.tensor_add(out=cred, in0=f[:, 0:2, :], in1=f[:, 2:4, :])

            # --- iou gates: 3 m-pair phases, weights shared across the
            # block's tiles ---
            gates_full = mid.tile([128, 6, 2 * TILE_N], FP16, tag="g", name="g")
            gates = gates_full[:, :, :w]
            for mp in range(3):
                pss = [
                    psp.tile([128, 2, TILE_N], F32, tag="ps", name="ps")
                    for _ in range(jw)
                ]
                for j in range(jw):
                    nj = cols[j].stop - cols[j].start
                    for m2 in range(2):
                        m = 2 * mp + m2
                        ms = slice(128 * m, 128 * (m + 1))
                        if br == 0:
                            for k in range(3):
                                nc.tensor.matmul(
                                    pss[j][:, m2, :nj], Wn_sb[:, k, ms],
                                    xt[:, k, cols[j]],
                                    start=(k == 0), stop=False,
                                )
                            for k in range(4):
                                nc.tensor.matmul(
                                    pss[j][:, m2, :nj], Un_sb[:, k, ms],
                                    ht[:, k, cols[j]],
                                    start=False, stop=(k == 3),
                                )
                        else:
                            for k in range(3):
                                nc.tensor.matmul(
                                    pss[j][:, m2, :nj], Ws_sb[:, k, ms],
                                    xt[:, k, cols[j]],
                                    start=(k == 0), stop=False,
                                )
                            for k in range(2):
                                nc.tensor.matmul(
                                    pss[j][:, m2, :nj], Us_sb[:, k, ms],
                                    htild[:, k, cols[j]],
                                    start=False, stop=(k == 1),
                                )
                for j in range(jw):
                    nj = cols[j].stop - cols[j].start
                    nc.scalar.activation(
                        out=gates[:, 2 * mp : 2 * mp + 2,
                                  j * TILE_N : j * TILE_N + nj],
                        in_=pss[j][:, :, :nj],
                        func=TANH if mp == 2 else SIG,
                    )

            # c = sig(i)*tanh(u) + cred (into gates[:,0:2])
            nc.vector.tensor_mul(
                out=gates[:, 0:2, :], in0=gates[:, 0:2, :], in1=gates[:, 4:6, :]
            )
            nc.vector.tensor_add(out=gates[:, 0:2, :], in0=gates[:, 0:2, :], in1=cred)
            nc.gpsimd.dma_start(out=cOT_v[:, :, n0 : n0 + w], in_=gates[:, 0:2, :])
            # h = sig(o)*tanh(c) (tanh into gates[:,4:6], h into gates[:,2:4])
            nc.scalar.activation(out=gates[:, 4:6, :], in_=gates[:, 0:2, :], func=TANH)
            nc.vector.tensor_mul(
                out=gates[:, 2:4, :], in0=gates[:, 2:4, :], in1=gates[:, 4:6, :]
            )
            nc.gpsimd.dma_start(out=hOT_v[:, :, n0 : n0 + w], in_=gates[:, 2:4, :])

        # --- macro schedule: two type-1 singles (cheap weight prefix
        # unblocks the PE early), then all type-0, then remaining type-1.
        # Counts are in 256-node halves; a segment's odd tail half-tile is
        # processed as a 256-node tile at the segment end. ---
        HALF = TILE_N // 2
        T0, half0 = divmod(T0h, 2)
        T1, half1 = divmod(T1h, 2)
        s1 = min(2, T1)
        base1 = T0h * HALF
        entries = []
        if s1:
            entries.append((1, base1, s1, half1, True))
        if T0 or half0:
            entries.append((0, 0, T0, half0, False))
        if (T1 - s1) or half1:
            entries.append((1, base1 + s1 * TILE_N, T1 - s1, half1, False))
        for ei, (br, base, T, half, singles) in enumerate(entries):
            starts = []
            g = 0
            while g < T:
                nt = 1 if singles else min(4, T - g)
                starts.append((g, nt))
                g += nt
            if half and not singles:
                # tack the 256-node tail tile onto the last macro (or its own)
                if starts and starts[-1][1] < 4:
                    starts[-1] = (starts[-1][0], starts[-1][1] + 0.5)
                else:
                    starts.append((g, 0.5))
            for gi, (g, ntf) in enumerate(starts):
                nt = int(ntf)
                tail = ntf != nt          # has a trailing 256-node half tile
                n0 = base + g * TILE_N
                w = nt * TILE_N + (HALF if tail else 0)
                nt_eff = nt + (1 if tail else 0)
                head = ei == 0 and gi == 0
                xt_full = io.tile([128, 3, MACRO], FP16, tag="xt", name="xt")
                xt = xt_full[:, :, :w]
                xeng = nc.gpsimd if head else nc.sync
                for k in range(3):
                    xeng.dma_start(out=xt[:, k, :], in_=xT_v[:, k, n0 : n0 + w])
                ht_full = io.tile([128, 4, MACRO], FP16, tag="ht", name="ht")
                ht = ht_full[:, :, :w]
                nc.sync.dma_start(out=ht, in_=hT_v[:, :, n0 : n0 + w])
                ct_full = io.tile([128, 4, MACRO], FP16, tag="ct", name="ct")
                ct = ct_full[:, :, :w]
                (nc.gpsimd if head else nc.scalar).dma_start(
                    out=ct, in_=cT_v[:, :, n0 : n0 + w]
                )
                if (ei == 0 and gi == len(starts) - 1) or br == 0:
                    load_br0_weights()

                htild = None
                if br == 1:
                    htild_full = io.tile(
                        [128, 2, MACRO], FP16, tag="htild", name="htild"
                    )
                    htild = htild_full[:, :, :w]
                    nc.vector.tensor_add(
                        out=htild, in0=ht[:, 0:2, :], in1=ht[:, 2:4, :]
                    )

                last = ei == len(entries) - 1 and gi == len(starts) - 1
                j0 = 0
                while j0 < nt_eff:
                    jw = 1 if (last and j0 >= nt_eff - 2) else min(2, nt_eff - j0)
                    ncol = HALF if (tail and j0 + jw == nt_eff) else TILE_N
                    do_block(br, xt, ht, ct, htild, j0, n0 + j0 * TILE_N, jw,
                             ncol)
                    j0 += jw

    nc.compile()
    _PROGRAM_CACHE[key] = nc
    return nc


def kernel(x, h_child, c_child, t, W_iou, U_iou, b_iou, U_f_w, U_f_b,
           W_iou_s, U_iou_s, b_iou_s, U_f_s_w, U_f_s_b):
    global LAST_EXEC_NS
    x = np.asarray(x, dtype=np.float32)
    h_child = np.asarray(h_child, dtype=np.float32)
    c_child = np.asarray(c_child, dtype=np.float32)
    t = np.asarray(t)
    n = x.shape[0]

    # --- host partition: equal per-core type counts, padded to tiles ---
    idx0 = np.flatnonzero(t == 0)
    idx1 = np.flatnonzero(t != 0)
    n0, n1 = len(idx0), len(idx1)

    def pad_split(idx, cnt):
        if cnt == 0:
            return np.zeros((CORES, 0), dtype=np.int64), 0
        per = _round_up(-(-cnt // CORES), TILE_N // 2)
        padded = np.concatenate(
            [idx, np.full(CORES * per - cnt, idx[-1], dtype=idx.dtype)]
        )
        return padded.reshape(CORES, per).astype(np.int64), per

    chunks0, P0 = pad_split(idx0, n0)
    chunks1, P1 = pad_split(idx1, n1)

    nc = _build_program(P0 // (TILE_N // 2), P1 // (TILE_N // 2))

    hc2 = h_child.reshape(n, 2 * H)
    cc2 = c_child.reshape(n, 2 * H)

    def bias_tile(v, m):
        return np.ascontiguousarray(
            np.asarray(v, np.float32).reshape(-1)[: 128 * m].reshape(m, 128).T
        )

    def w_with_bias(W, b):
        # [XP, 768] = W^T with bias as row 300, zero rows 301..383
        out = np.zeros((XP, 3 * H), dtype=NP_FP16)
        out[:X] = np.asarray(W, np.float32).T.astype(NP_FP16)
        out[X] = np.asarray(b, np.float32).reshape(-1).astype(NP_FP16)
        return out

    wmap = {
        "WnT": w_with_bias(W_iou, b_iou),
        "UnT": np.ascontiguousarray(np.asarray(U_iou, np.float32).T).astype(NP_FP16),
        "UfwT": np.ascontiguousarray(np.asarray(U_f_w, np.float32).T).astype(NP_FP16),
        "WsT": w_with_bias(W_iou_s, b_iou_s),
        "UsT": np.ascontiguousarray(np.asarray(U_iou_s, np.float32).T).astype(NP_FP16),
        "UfswT": np.ascontiguousarray(np.asarray(U_f_s_w, np.float32).T).astype(NP_FP16),
        "bias_fn": bias_tile(U_f_b, 4),
        "bias_fs": bias_tile(U_f_s_b, 2),
    }

    in_maps = []
    for i in range(CORES):
        I = np.concatenate([chunks0[i], chunks1[i]])
        xTi = np.zeros((XP, len(I)), dtype=NP_FP16)
        xTi[:X] = x[I].T.astype(NP_FP16)
        xTi[X] = 1.0
        m = dict(wmap)
        m["xT"] = xTi
        m["hT"] = hc2[I].T.astype(NP_FP16)
        m["cT"] = cc2[I].T.astype(NP_FP16)
        in_maps.append(m)

    res = bass_utils.run_bass_kernel_spmd(
        nc, in_maps, core_ids=list(range(CORES)), trace=TRACE
    )
    LAST_EXEC_NS = res.exec_time_ns

    # --- scatter back ---
    h_out = np.empty((n, H), dtype=np.float32)
    c_out = np.empty((n, H), dtype=np.float32)
    if n0:
        h0 = np.concatenate(
            [res.results[i]["hOT"][:, :P0].T.astype(np.float32) for i in range(CORES)]
        )
        c0 = np.concatenate(
            [res.results[i]["cOT"][:, :P0].T.astype(np.float32) for i in range(CORES)]
        )
        h_out[idx0] = h0[:n0]
        c_out[idx0] = c0[:n0]
    if n1:
        h1 = np.concatenate(
            [res.results[i]["hOT"][:, P0:].T.astype(np.float32) for i in range(CORES)]
        )
        c1 = np.concatenate(
            [res.results[i]["cOT"][:, P0:].T.astype(np.float32) for i in range(CORES)]
        )
        h_out[idx1] = h1[:n1]
        c_out[idx1] = c1[:n1]
    return h_out, c_out


# revision 10
# speedup vs baseline: 1.2047x; 1.0006x over previous
"""MixTreeLSTMCell Trainium2 kernel (8 NeuronCores, SPMD).

The cell evaluates one of two branches per node (t in {0,1}); the host
partitions nodes by type so each core runs two static branch segments with
no per-node select, on feature-major fp16 operands. The device program is
tensor-engine-bound (~1574 fp16 matmuls/core at ~218ns), so the design
keeps the PE gap-free:
- x^T zero-padded on host from 301 to 384 contraction rows: every matmul
  is a full 128-partition instruction (sub-128-row matmuls measured 1.5x
  slower per moving row on HW).
- Node tiles processed in blocks of two 512-node tiles with bank-stable
  PSUM targeting (runs of consecutive matmuls into one bank), rotating
  through one 8-bank PSUM pool deep enough that ACT drains never stall
  the PE.
- Macro schedule: two type-1 single-tile warmups first (their small
  f/iou weights load fastest, unblocking the PE at ~10us), then all
  type-0 macros, then the remaining type-1; type-0 weights stream in
  lazily behind the warmups' data in ring-queue need order.
- Per-core type counts padded to 256-node halves (segment tails run one
  256-wide tile) to minimize padded compute.
- fp16 outputs; iou bias folded into the matmul via a ones row; f-gate
  biases applied by the scalar engine's free bias; the elementwise chain
  runs in place inside the gates tile and h/c are DMA'd straight out of
  it on the SWDGE ring.
"""

from contextlib import ExitStack

import numpy as np

import concourse.bacc as bacc
import concourse.tile as tile
from concourse import mybir
from concourse import bass_utils

F32 = mybir.dt.float32
FP16 = mybir.dt.float16
NP_FP16 = np.float16

N_NODES = 131072
X = 300
XP = 384              # x rows + ones(bias) row at 300, zero-padded to 3*128
H = 256
CORES = 8
TILE_N = 512          # nodes per matmul tile (max moving free dim)
MACRO = 4 * TILE_N    # nodes per DMA macro tile

TRACE = False
LAST_EXEC_NS = None

_PROGRAM_CACHE = {}


def _round_up(v, m):
    return (v + m - 1) // m * m


def _build_program(T0h, T1h):
    """T0h/T1h: per-core type-0/1 node counts in 256-node half-tile units."""
    key = (T0h, T1h)
    if key in _PROGRAM_CACHE:
        return _PROGRAM_CACHE[key]

    Nc = (T0h + T1h) * (TILE_N // 2)
    nc = bacc.Bacc("TRN2", target_bir_lowering=False, debug=False)

    xT = nc.dram_tensor("xT", [XP, Nc], FP16, kind="ExternalInput").ap()
    hT = nc.dram_tensor("hT", [2 * H, Nc], FP16, kind="ExternalInput").ap()
    cT = nc.dram_tensor("cT", [2 * H, Nc], FP16, kind="ExternalInput").ap()

    WnT = nc.dram_tensor("WnT", [XP, 3 * H], FP16, kind="ExternalInput").ap()
    UnT = nc.dram_tensor("UnT", [2 * H, 3 * H], FP16, kind="ExternalInput").ap()
    UfwT = nc.dram_tensor("UfwT", [2 * H, 2 * H], FP16, kind="ExternalInput").ap()
    WsT = nc.dram_tensor("WsT", [XP, 3 * H], FP16, kind="ExternalInput").ap()
    UsT = nc.dram_tensor("UsT", [H, 3 * H], FP16, kind="ExternalInput").ap()
    UfswT = nc.dram_tensor("UfswT", [H, H], FP16, kind="ExternalInput").ap()

    bias_fn = nc.dram_tensor("bias_fn", [128, 4], F32, kind="ExternalInput").ap()
    bias_fs = nc.dram_tensor("bias_fs", [128, 2], F32, kind="ExternalInput").ap()

    hOT = nc.dram_tensor("hOT", [H, Nc], FP16, kind="ExternalOutput").ap()
    cOT = nc.dram_tensor("cOT", [H, Nc], FP16, kind="ExternalOutput").ap()

    xT_v = xT.rearrange("(ko p) n -> p ko n", p=128)
    hT_v = hT.rearrange("(ko p) n -> p ko n", p=128)
    cT_v = cT.rearrange("(ko p) n -> p ko n", p=128)
    hOT_v = hOT.rearrange("(ko p) n -> p ko n", p=128)
    cOT_v = cOT.rearrange("(ko p) n -> p ko n", p=128)
    WnT_v = WnT.rearrange("(ko p) m -> p ko m", p=128)
    WsT_v = WsT.rearrange("(ko p) m -> p ko m", p=128)
    UnT_v = UnT.rearrange("(ko p) m -> p ko m", p=128)
    UfwT_v = UfwT.rearrange("(ko p) m -> p ko m", p=128)
    UsT_v = UsT.rearrange("(ko p) m -> p ko m", p=128)
    UfswT_v = UfswT.rearrange("(ko p) m -> p ko m", p=128)

    SIG = mybir.ActivationFunctionType.Sigmoid
    TANH = mybir.ActivationFunctionType.Tanh

    with tile.TileContext(nc) as tc, ExitStack() as stack:
        wp = stack.enter_context(tc.tile_pool(name="w", bufs=1))
        io = stack.enter_context(tc.tile_pool(name="io", bufs=2))
        mid = stack.enter_context(tc.tile_pool(name="mid", bufs=2))
        psp = stack.enter_context(tc.tile_pool(name="psp", bufs=4, space="PSUM"))

        # --- resident weights in need order: the two type-1 warmup singles
        # run first (Ufsw, Ws, Us), then type-0 (Ufw, Wn, Un); spread across
        # the three DMA rings so no single ring serializes the head ---
        Ufsw_sb = wp.tile([128, 2, H], FP16)
        nc.sync.dma_start(out=Ufsw_sb, in_=UfswT_v)
        bfs_sb = wp.tile([128, 2], F32)
        nc.sync.dma_start(out=bfs_sb, in_=bias_fs)
        bfn_sb = wp.tile([128, 4], F32)
        nc.gpsimd.dma_start(out=bfn_sb, in_=bias_fn)
        Ws_sb = wp.tile([128, 3, 3 * H], FP16)
        Us_sb = wp.tile([128, 2, 3 * H], FP16)
        nc.scalar.dma_start(out=Ws_sb[:, 0, :], in_=WsT_v[:, 0, :])
        nc.scalar.dma_start(out=Ws_sb[:, 1, :], in_=WsT_v[:, 1, :])
        nc.scalar.dma_start(out=Ws_sb[:, 2, :], in_=WsT_v[:, 2, :])
        nc.scalar.dma_start(out=Us_sb[:, 0, :], in_=UsT_v[:, 0, :])
        nc.scalar.dma_start(out=Us_sb[:, 1, :], in_=UsT_v[:, 1, :])
        # type-0 weights: allocated here, but their DMAs are emitted only
        # after the warmup singles' data loads so the ring queues serve the
        # head in true need order.
        Ufw_sb = wp.tile([128, 4, 2 * H], FP16)
        Wn_sb = wp.tile([128, 3, 3 * H], FP16)
        Un_sb = wp.tile([128, 4, 3 * H], FP16)
        br0_fired = [False]

        def load_br0_weights():
            if br0_fired[0]:
                return
            br0_fired[0] = True
            nc.sync.dma_start(out=Ufw_sb[:, 0:2, :], in_=UfwT_v[:, 0:2, :])
            nc.gpsimd.dma_start(out=Ufw_sb[:, 2:4, :], in_=UfwT_v[:, 2:4, :])
            nc.sync.dma_start(out=Wn_sb[:, 0, :], in_=WnT_v[:, 0, :])
            nc.scalar.dma_start(out=Wn_sb[:, 1, :], in_=WnT_v[:, 1, :])
            nc.gpsimd.dma_start(out=Wn_sb[:, 2, :], in_=WnT_v[:, 2, :])
            nc.sync.dma_start(out=Un_sb[:, 0, :], in_=UnT_v[:, 0, :])
            nc.scalar.dma_start(out=Un_sb[:, 1, :], in_=UnT_v[:, 1, :])
            nc.gpsimd.dma_start(out=Un_sb[:, 2, :], in_=UnT_v[:, 2, :])
            nc.sync.dma_start(out=Un_sb[:, 3, :], in_=UnT_v[:, 3, :])

        def do_block(br, xt, ht, ct, htild, j0, n0, jw, ncol=TILE_N):
            """Process jw (1 or 2) node tiles starting at macro tile j0.
            n0 = DRAM node offset; ncol<=512 nodes per tile (last tile of a
            segment may be a 256-node half tile)."""
            w = (jw - 1) * TILE_N + ncol
            cols = [slice((j0 + j) * TILE_N,
                          (j0 + j) * TILE_N + (TILE_N if j < jw - 1 else ncol))
                    for j in range(jw)]
            span = slice(j0 * TILE_N, j0 * TILE_N + w)

            f_full = mid.tile([128, 4, 2 * TILE_N], FP16, tag="f", name="f")
            f = f_full[:, :, :w]

            # --- forget gates ---
            if br == 0:
                for m in range(4):
                    ps = psp.tile([128, 2, TILE_N], F32, tag="ps", name="ps")
                    for j in range(jw):
                        nj = cols[j].stop - cols[j].start
                        for k in range(4):
                            nc.tensor.matmul(
                                ps[:, j, :nj],
                                Ufw_sb[:, k, 128 * m : 128 * (m + 1)],
                                ht[:, k, cols[j]],
                                start=(k == 0),
                                stop=(k == 3),
                            )
                    if ncol == TILE_N:
                        nc.scalar.activation(
                            out=f[:, m, :],
                            in_=ps[:, :jw, :].rearrange("p a b -> p (a b)"),
                            func=SIG,
                            bias=bfn_sb[:, m : m + 1],
                        )
                    else:
                        for j in range(jw):
                            nj = cols[j].stop - cols[j].start
                            nc.scalar.activation(
                                out=f[:, m, j * TILE_N : j * TILE_N + nj],
                                in_=ps[:, j, :nj],
                                func=SIG,
                                bias=bfn_sb[:, m : m + 1],
                            )
            else:
                for child in range(2):
                    for m in range(2):
                        ps = psp.tile([128, 2, TILE_N], F32, tag="ps", name="ps")
                        for j in range(jw):
                            nj = cols[j].stop - cols[j].start
                            for k in range(2):
                                nc.tensor.matmul(
                                    ps[:, j, :nj],
                                    Ufsw_sb[:, k, 128 * m : 128 * (m + 1)],
                                    ht[:, 2 * child + k, cols[j]],
                                    start=(k == 0),
                                    stop=(k == 1),
                                )
                        if ncol == TILE_N:
                            nc.scalar.activation(
                                out=f[:, 2 * child + m, :],
                                in_=ps[:, :jw, :].rearrange("p a b -> p (a b)"),
                                func=SIG,
                                bias=bfs_sb[:, m : m + 1],
                            )
                        else:
                            for j in range(jw):
                                nj = cols[j].stop - cols[j].start
                                nc.scalar.activation(
                                    out=f[:, 2 * child + m,
                                          j * TILE_N : j * TILE_N + nj],
                                    in_=ps[:, j, :nj],
                                    func=SIG,
                                    bias=bfs_sb[:, m : m + 1],
                                )

            # prod = f * c_child (in place); cred = child0 + child1
            nc.vector.tensor_mul(out=f, in0=f, in1=ct[:, :, span])
            cred_full = mid.tile([128, 2, 2 * TILE_N], F32, tag="cred", name="cred")
            cred = cred_full[:, :, :w]
            nc.vector.tensor_add(out=cred, in0=f[:, 0:2, :], in1=f[:, 2:4, :])

            # --- iou gates: 3 m-pair phases, weights shared across the
            # block's tiles ---
            gates_full = mid.tile([128, 6, 2 * TILE_N], FP16, tag="g", name="g")
            gates = gates_full[:, :, :w]
            for mp in range(3):
                pss = [
                    psp.tile([128, 2, TILE_N], F32, tag="ps", name="ps")
                    for _ in range(jw)
                ]
                for j in range(jw):
                    nj = cols[j].stop - cols[j].start
                    for m2 in range(2):
                        m = 2 * mp + m2
                        ms = slice(128 * m, 128 * (m + 1))
                        if br == 0:
                            for k in range(3):
                                nc.tensor.matmul(
                                    pss[j][:, m2, :nj], Wn_sb[:, k, ms],
                                    xt[:, k, cols[j]],
                                    start=(k == 0), stop=False,
                                )
                            for k in range(4):
                                nc.tensor.matmul(
                                    pss[j][:, m2, :nj], Un_sb[:, k, ms],
                                    ht[:, k, cols[j]],
                                    start=False, stop=(k == 3),
                                )
                        else:
                            for k in range(3):
                                nc.tensor.matmul(
                                    pss[j][:, m2, :nj], Ws_sb[:, k, ms],
                                    xt[:, k, cols[j]],
                                    start=(k == 0), stop=False,
                                )
                            for k in range(2):
                                nc.tensor.matmul(
                                    pss[j][:, m2, :nj], Us_sb[:, k, ms],
                                    htild[:, k, cols[j]],
                                    start=False, stop=(k == 1),
                                )
                for j in range(jw):
                    nj = cols[j].stop - cols[j].start
                    nc.scalar.activation(
                        out=gates[:, 2 * mp : 2 * mp + 2,
                                  j * TILE_N : j * TILE_N + nj],
                        in_=pss[j][:, :, :nj],
                        func=TANH if mp == 2 else SIG,
                    )

            # c = sig(i)*tanh(u) + cred (into gates[:,0:2])
            nc.vector.tensor_mul(
                out=gates[:, 0:2, :], in0=gates[:, 0:2, :], in1=gates[:, 4:6, :]
            )
            nc.vector.tensor_add(out=gates[:, 0:2, :], in0=gates[:, 0:2, :], in1=cred)
            nc.gpsimd.dma_start(out=cOT_v[:, :, n0 : n0 + w], in_=gates[:, 0:2, :])
            # h = sig(o)*tanh(c) (tanh into gates[:,4:6], h into gates[:,2:4])
            nc.scalar.activation(out=gates[:, 4:6, :], in_=gates[:, 0:2, :], func=TANH)
            nc.vector.tensor_mul(
                out=gates[:, 2:4, :], in0=gates[:, 2:4, :], in1=gates[:, 4:6, :]
            )
            nc.gpsimd.dma_start(out=hOT_v[:, :, n0 : n0 + w], in_=gates[:, 2:4, :])

        # --- macro schedule: two type-1 singles (cheap weight prefix
        # unblocks the PE early), then all type-0, then remaining type-1.
        # Counts are in 256-node halves; a segment's odd tail half-tile is
        # processed as a 256-node tile at the segment end. ---
        HALF = TILE_N // 2
        T0, half0 = divmod(T0h, 2)
        T1, half1 = divmod(T1h, 2)
        s1 = min(3, T1)
        base1 = T0h * HALF
        entries = []
        if s1:
            entries.append((1, base1, s1, half1, True))
        if T0 or half0:
            entries.append((0, 0, T0, half0, False))
        if (T1 - s1) or half1:
            entries.append((1, base1 + s1 * TILE_N, T1 - s1, half1, False))
        for ei, (br, base, T, half, singles) in enumerate(entries):
            starts = []
            g = 0
            while g < T:
                nt = 1 if singles else min(4, T - g)
                starts.append((g, nt))
                g += nt
            if half and not singles:
                # tack the 256-node tail tile onto the last macro (or its own)
                if starts and starts[-1][1] < 4:
                    starts[-1] = (starts[-1][0], starts[-1][1] + 0.5)
                else:
                    starts.append((g, 0.5))
            for gi, (g, ntf) in enumerate(starts):
                nt = int(ntf)
                tail = ntf != nt          # has a trailing 256-node half tile
                n0 = base + g * TILE_N
                w = nt * TILE_N + (HALF if tail else 0)
                nt_eff = nt + (1 if tail else 0)
                head = ei == 0 and gi == 0
                xt_full = io.tile([128, 3, MACRO], FP16, tag="xt", name="xt")
                xt = xt_full[:, :, :w]
                ht_full = io.tile([128, 4, MACRO], FP16, tag="ht", name="ht")
                ht = ht_full[:, :, :w]
                ct_full = io.tile([128, 4, MACRO], FP16, tag="ct", name="ct")
                ct = ct_full[:, :, :w]
                if head:
                    # f-gates consume ht k0..k3 first, then iou needs xt;
                    # ct is only read by the vector engine later.
                    nc.gpsimd.dma_start(
                        out=ht[:, 2:4, :], in_=hT_v[:, 2:4, n0 : n0 + w]
                    )
                    for k in range(3):
                        nc.gpsimd.dma_start(
                            out=xt[:, k, :], in_=xT_v[:, k, n0 : n0 + w]
                        )
                    nc.sync.dma_start(
                        out=ht[:, 0:2, :], in_=hT_v[:, 0:2, n0 : n0 + w]
                    )
                    nc.sync.dma_start(
                        out=ct[:, 0:2, :], in_=cT_v[:, 0:2, n0 : n0 + w]
                    )
                    nc.scalar.dma_start(
                        out=ct[:, 2:4, :], in_=cT_v[:, 2:4, n0 : n0 + w]
                    )
                else:
                    for k in range(3):
                        nc.sync.dma_start(
                            out=xt[:, k, :], in_=xT_v[:, k, n0 : n0 + w]
                        )
                    nc.sync.dma_start(out=ht, in_=hT_v[:, :, n0 : n0 + w])
                    nc.scalar.dma_start(out=ct, in_=cT_v[:, :, n0 : n0 + w])
                if (ei == 0 and gi == len(starts) - 1) or br == 0:
                    load_br0_weights()

                htild = None
                if br == 1:
                    htild_full = io.tile(
                        [128, 2, MACRO], FP16, tag="htild", name="htild"
                    )
                    htild = htild_full[:, :, :w]
                    nc.vector.tensor_add(
                        out=htild, in0=ht[:, 0:2, :], in1=ht[:, 2:4, :]
                    )

                last = ei == len(entries) - 1 and gi == len(starts) - 1
                j0 = 0
                while j0 < nt_eff:
                    jw = 1 if (last and j0 >= nt_eff - 2) else min(2, nt_eff - j0)
                    ncol = HALF if (tail and j0 + jw == nt_eff) else TILE_N
                    do_block(br, xt, ht, ct, htild, j0, n0 + j0 * TILE_N, jw,
                             ncol)
                    j0 += jw

    nc.compile()
    _PROGRAM_CACHE[key] = nc
    return nc


def kernel(x, h_child, c_child, t, W_iou, U_iou, b_iou, U_f_w, U_f_b,
           W_iou_s, U_iou_s, b_iou_s, U_f_s_w, U_f_s_b):
    global LAST_EXEC_NS
    x = np.asarray(x, dtype=np.float32)
    h_child = np.asarray(h_child, dtype=np.float32)
    c_child = np.asarray(c_child, dtype=np.float32)
    t = np.asarray(t)
    n = x.shape[0]

    # --- host partition: equal per-core type counts, padded to tiles ---
    idx0 = np.flatnonzero(t == 0)
    idx1 = np.flatnonzero(t != 0)
    n0, n1 = len(idx0), len(idx1)

    def pad_split(idx, cnt):
        if cnt == 0:
            return np.zeros((CORES, 0), dtype=np.int64), 0
        per = _round_up(-(-cnt // CORES), TILE_N // 2)
        padded = np.concatenate(
            [idx, np.full(CORES * per - cnt, idx[-1], dtype=idx.dtype)]
        )
        return padded.reshape(CORES, per).astype(np.int64), per

    chunks0, P0 = pad_split(idx0, n0)
    chunks1, P1 = pad_split(idx1, n1)

    nc = _build_program(P0 // (TILE_N // 2), P1 // (TILE_N // 2))

    hc2 = h_child.reshape(n, 2 * H)
    cc2 = c_child.reshape(n, 2 * H)

    def bias_tile(v, m):
        return np.ascontiguousarray(
            np.asarray(v, np.float32).reshape(-1)[: 128 * m].reshape(m, 128).T
        )

    def w_with_bias(W, b):
        # [XP, 768] = W^T with bias as row 300, zero rows 301..383
        out = np.zeros((XP, 3 * H), dtype=NP_FP16)
        out[:X] = np.asarray(W, np.float32).T.astype(NP_FP16)
        out[X] = np.asarray(b, np.float32).reshape(-1).astype(NP_FP16)
        return out

    wmap = {
        "WnT": w_with_bias(W_iou, b_iou),
        "UnT": np.ascontiguousarray(np.asarray(U_iou, np.float32).T).astype(NP_FP16),
        "UfwT": np.ascontiguousarray(np.asarray(U_f_w, np.float32).T).astype(NP_FP16),
        "WsT": w_with_bias(W_iou_s, b_iou_s),
        "UsT": np.ascontiguousarray(np.asarray(U_iou_s, np.float32).T).astype(NP_FP16),
        "UfswT": np.ascontiguousarray(np.asarray(U_f_s_w, np.float32).T).astype(NP_FP16),
        "bias_fn": bias_tile(U_f_b, 4),
        "bias_fs": bias_tile(U_f_s_b, 2),
    }

    in_maps = []
    for i in range(CORES):
        I = np.concatenate([chunks0[i], chunks1[i]])
        xTi = np.zeros((XP, len(I)), dtype=NP_FP16)
        xTi[:X] = x[I].T.astype(NP_FP16)
        xTi[X] = 1.0
        m = dict(wmap)
        m["xT"] = xTi
        m["hT"] = hc2[I].T.astype(NP_FP16)
        m["cT"] = cc2[I].T.astype(NP_FP16)
        in_maps.append(m)

    res = bass_utils.run_bass_kernel_spmd(
        nc, in_maps, core_ids=list(range(CORES)), trace=TRACE
    )
    LAST_EXEC_NS = res.exec_time_ns

    # --- scatter back ---
    h_out = np.empty((n, H), dtype=np.float32)
    c_out = np.empty((n, H), dtype=np.float32)
    if n0:
        h0 = np.concatenate(
            [res.results[i]["hOT"][:, :P0].T.astype(np.float32) for i in range(CORES)]
        )
        c0 = np.concatenate(
            [res.results[i]["cOT"][:, :P0].T.astype(np.float32) for i in range(CORES)]
        )
        h_out[idx0] = h0[:n0]
        c_out[idx0] = c0[:n0]
    if n1:
        h1 = np.concatenate(
            [res.results[i]["hOT"][:, P0:].T.astype(np.float32) for i in range(CORES)]
        )
        c1 = np.concatenate(
            [res.results[i]["cOT"][:, P0:].T.astype(np.float32) for i in range(CORES)]
        )
        h_out[idx1] = h1[:n1]
        c_out[idx1] = c1[:n1]
    return h_out, c_out
